# revision 1
# baseline (speedup 1.0000x reference)
"""Multi-head attention (B=4, S=2048, D=1024, H=16) on 8 trn2 NeuronCores.

Sharding: 8 cores = 4 batches x 2 head-groups. Core c handles batch c//2 and
heads [8g, 8g+8) where g = c%2 (tensor-parallel: Wq/Wk/Wv column-sliced,
Wo row-sliced). Each core returns a partial output [S, D]; the host sums the
two head-group partials per batch.

Per-core dataflow (everything stays transposed until the output projection):
  xT tiles (PE transpose) -> Q.T/K.T = W.T @ X.T (bf16), V natural (bf16,
  ones column appended) -> scores.T = K @ Q.T -> exp+mask+scale in one ACT op
  -> ctxU.T = V'.T @ expS.T (last row = softmax denominator) -> normalize ->
  out = ctx.T.T @ Wo (fp32r) + bo.
"""

import sys

if "/opt/trn_rl_repo" not in sys.path:
    sys.path.append("/opt/trn_rl_repo")

import numpy as np

import concourse.bass as bass
import concourse.bacc as bacc
import concourse.tile as tile
from concourse import mybir
from concourse.bass import ts
from concourse.masks import make_identity

F32 = mybir.dt.float32
F32R = mybir.dt.float32r
BF16 = mybir.dt.bfloat16
I32 = mybir.dt.int32
EXP = mybir.ActivationFunctionType.Exp

P = 128


def build_nc(S=2048, D=1024, DL=512, HD=64):
    """Build the per-core Bass program. DL = local output dim (heads*HD)."""
    ST = S // P  # token tiles
    KD = D // P  # contraction tiles over D
    MT = DL // P  # local d-col tiles
    HL = DL // HD  # local heads
    HPT = P // HD  # heads per 128-partition tile (2)
    NCH = min(512, S)  # projection token-chunk
    QS = min(1024, S)  # attention q superchunk (<=2 psum banks)
    QH = min(512, QS)  # one-bank half
    NH = QS // QH
    NQ = S // QS  # q-superchunks
    OC = min(512, D)  # out-proj col chunk
    scale = float(1.0 / (np.sqrt(np.float32(HD)) + 1e-8))

    nc = bacc.Bacc("TRN2", target_bir_lowering=False, debug=False)

    xq = nc.dram_tensor("xq", [S, D], F32, kind="ExternalInput")
    xk = nc.dram_tensor("xk", [S, D], F32, kind="ExternalInput")
    xv = nc.dram_tensor("xv", [S, D], F32, kind="ExternalInput")
    msk = nc.dram_tensor("msk", [P, ST], I32, kind="ExternalInput")
    wq = nc.dram_tensor("wq", [D, DL], F32, kind="ExternalInput")
    wk = nc.dram_tensor("wk", [D, DL], F32, kind="ExternalInput")
    wv = nc.dram_tensor("wv", [D, DL], F32, kind="ExternalInput")
    wo = nc.dram_tensor("wo", [DL, D], F32, kind="ExternalInput")
    bq = nc.dram_tensor("bq", [P, MT], F32, kind="ExternalInput")
    bk = nc.dram_tensor("bk", [P, MT], F32, kind="ExternalInput")
    bv = nc.dram_tensor("bv", [1, DL], F32, kind="ExternalInput")
    bo = nc.dram_tensor("bo", [1, D], F32, kind="ExternalInput")
    out = nc.dram_tensor("out", [S, D], F32, kind="ExternalOutput")

    with tile.TileContext(nc) as tc, nc.allow_low_precision("fp32r matmul operands are rounded by design"):
        with (
            tc.tile_pool(name="pers", bufs=1) as pers,
            tc.tile_pool(name="wpool", bufs=1) as wpool,
            tc.tile_pool(name="xnat", bufs=3) as xnat_pool,
            tc.tile_pool(name="xt", bufs=KD + 1) as xt_pool,
            tc.tile_pool(name="xtc", bufs=6) as xtc_pool,
            tc.tile_pool(name="exp", bufs=10) as ex_pool,
            tc.tile_pool(name="osb", bufs=2) as osb_pool,
            tc.tile_pool(name="small", bufs=2) as small,
        ):
            # ---- constants ----
            ident = pers.tile([P, P], F32, tag="ident")
            make_identity(nc, ident[:])
            ones0 = pers.tile([1, P], F32, tag="ones0")
            nc.gpsimd.memset(ones0[:], 1.0)
            ones = pers.tile([1, P], F32R, tag="ones")
            nc.vector.tensor_copy(out=ones[:], in_=ones0[:])

            mi = pers.tile([P, ST], I32, tag="mi")
            nc.sync.dma_start(mi[:], msk[:, :])
            mf = pers.tile([P, ST], F32, tag="mf")
            nc.vector.tensor_copy(out=mf[:], in_=mi[:])
            mb = pers.tile([P, ST], F32, tag="mb")
            nc.vector.tensor_scalar_mul(mb[:], mf[:], -1.0e9)

            bqs = pers.tile([P, MT], F32, tag="bqs")
            nc.sync.dma_start(bqs[:], bq[:, :])
            bks = pers.tile([P, MT], F32, tag="bks")
            nc.sync.dma_start(bks[:], bk[:, :])
            bvstg = small.tile([1, D], F32, tag="bstg", name="bvstg")
            nc.sync.dma_start(bvstg[0:1, 0:DL], bv[:, :])
            bvs = pers.tile([1, DL], F32R, tag="bvs")
            nc.vector.tensor_copy(out=bvs[:], in_=bvstg[0:1, 0:DL])
            bostg = small.tile([1, D], F32, tag="bstg", name="bostg")
            nc.sync.dma_start(bostg[:], bo[:, :])
            bos = pers.tile([1, D], F32R, tag="bos")
            nc.vector.tensor_copy(out=bos[:], in_=bostg[:])
            bvb = pers.tile([P, DL], F32, tag="bvb")
            bob = pers.tile([P, D], F32, tag="bob")

            wstg0 = wpool.tile([P, MT, D], F32, tag="wstg", name="wstg0")
            nc.sync.dma_start(wstg0[:], wo.rearrange("(m p) n -> p m n", p=P))
            wos = pers.tile([P, MT, D], BF16, tag="wos")
            nc.vector.tensor_copy(out=wos[:], in_=wstg0[:])

            # persistent activation stores
            KT = [pers.tile([P, S], BF16, tag=f"kt{m}", name=f"kt{m}") for m in range(MT)]
            QT = [pers.tile([P, S], BF16, tag=f"qt{m}", name=f"qt{m}") for m in range(MT)]
            CT = [pers.tile([P, S], BF16, tag=f"ct{m}", name=f"ct{m}") for m in range(MT)]
            VP = [pers.tile([P, HL * (HD + 1)], BF16, tag=f"vp{t}", name=f"vp{t}") for t in range(ST)]
            for t in range(ST):
                nc.gpsimd.memset(VP[t][:], 1.0)

            def load_w(wdram):
                stg = wpool.tile([P, KD, DL], F32, tag="wstg", name="wstg")
                nc.sync.dma_start(stg[:], wdram.rearrange("(k p) n -> p k n", p=P))
                w = wpool.tile([P, KD, DL], F32R, tag="w", name="w")
                nc.vector.tensor_copy(out=w[:], in_=stg[:])
                return w

            def tpose(dst, src, tp_slot):
                """dst[128, 128] (SBUF) = src[128, 128].T via PE."""
                nc.tensor.transpose(tp_slot, src, ident[:])
                nc.vector.tensor_copy(out=dst, in_=tp_slot)

            def proj_units(xdram, wsb, bias_sb, dst_tiles, nch, tp_pool, acc_pool):
                """dst[m][:, nch-chunk] = (x @ w + b).T; yields at unit edges."""
                nt = NCH // P
                xts = [
                    xt_pool.tile([P, NCH], F32R, tag="xt", name="xt") for _ in range(KD)
                ]
                for i in range(nt):
                    xn = xnat_pool.tile([P, D], F32, tag="xnat")
                    nc.sync.dma_start(xn[:], xdram[ts(nch * nt + i, P), :])
                    tp4 = (tp_pool if i % 2 == 0 else acc_pool).tile(
                        [P, 4, P], F32, tag="tp" if i % 2 == 0 else "acc", name="tp4"
                    )
                    for kk in range(KD):
                        tpose(xts[kk][:, ts(i, P)], xn[:, ts(kk, P)], tp4[:, kk % 4, :])
                    yield
                for m in range(MT):
                    acc = acc_pool.tile([P, NCH], F32, tag="acc")
                    for kk in range(KD):
                        nc.tensor.matmul(
                            acc[:],
                            lhsT=wsb[:, kk, ts(m, P)],
                            rhs=xts[kk][:],
                            start=(kk == 0),
                            stop=(kk == KD - 1),
                        )
                    nc.vector.tensor_scalar_add(
                        dst_tiles[m][:, ts(nch, NCH)], acc[:], bias_sb[:, m : m + 1]
                    )
                    yield

            def proj_T(xdram, wsb, bias_sb, dst_tiles, nch, tp_pool, acc_pool):
                for _ in proj_units(
                    xdram, wsb, bias_sb, dst_tiles, nch, tp_pool, acc_pool
                ):
                    pass

            def vproj(wsb, tp_pool, acc_pool):
                """VP[t][:, h*(HD+1):+HD] = (xv @ wv + bv)[t-tile, h-slice]."""
                for t in range(ST):
                    xn = xnat_pool.tile([P, D], F32, tag="xnat")
                    nc.sync.dma_start(xn[:], xv[ts(t, P), :])
                    xts = []
                    tp4 = tp_pool.tile([P, 4, P], F32, tag="tp", name="tp4")
                    for kk in range(KD):
                        xt = xtc_pool.tile([P, P], F32R, tag="xtc")
                        tpose(xt[:], xn[:, ts(kk, P)], tp4[:, kk % 4, :])
                        xts.append(xt)
                    acc = acc_pool.tile([P, DL], F32, tag="acc")
                    for kk in range(KD):
                        nc.tensor.matmul(
                            acc[:],
                            lhsT=xts[kk][:],
                            rhs=wsb[:, kk, :],
                            start=(kk == 0),
                            stop=(kk == KD - 1),
                        )
                    for h in range(HL):
                        nc.vector.tensor_add(
                            VP[t][:, h * (HD + 1) : h * (HD + 1) + HD],
                            acc[:, ts(h, HD)],
                            bvb[:, ts(h, HD)],
                        )

            def attention(qq, sc_pool, cx_pool, tp_pool, filler=None, pump_every=8):
                it = 0
                pending = []  # deferred normalize work (recip/broadcast/mul)
                for hp in range(HL // HPT):  # head pairs share a KT/QT tile
                    for q5 in range(NH):
                        col0 = qq * QS + q5 * QH
                        cxs = [
                            cx_pool.tile([HD + 1, QH], F32, tag="cx", name="cx")
                            for _ in range(HPT)
                        ]
                        for kt in range(ST):
                            # one PSUM supertile holds both heads' score chunk;
                            # the two K=64 matmuls run concurrently (row groups
                            # 0-63 / 64-127), one ACT exp covers both
                            sc = sc_pool.tile([P, HPT * QH], F32, tag="sc")
                            for u in range(HPT):
                                mo = u * HD
                                nc.tensor.matmul(
                                    sc[:, ts(u, QH)],
                                    lhsT=KT[hp][mo : mo + HD, ts(kt, P)],
                                    rhs=QT[hp][mo : mo + HD, col0 : col0 + QH],
                                    start=True,
                                    stop=True,
                                )
                            ex = ex_pool.tile([P, HPT * QH], BF16, tag="ex")
                            nc.scalar.activation(
                                ex[:], sc[:], EXP, bias=mb[:, kt : kt + 1], scale=scale
                            )
                            for u in range(HPT):
                                h = hp * HPT + u
                                nc.tensor.matmul(
                                    cxs[u][:],
                                    lhsT=VP[kt][:, h * (HD + 1) : (h + 1) * (HD + 1)],
                                    rhs=ex[:, ts(u, QH)],
                                    start=(kt == 0),
                                    stop=(kt == ST - 1),
                                )
                            it += 1
                            if filler is not None and it % pump_every == 0:
                                next(filler, None)
                        prev_tails = pending
                        pending = []
                        for u in range(HPT):
                            mo = u * HD
                            # the cheap DVE copy (emitted now, ahead of the
                            # previous unit's reciprocals in DVE order) frees
                            # the ctx PSUM slot; recip/broadcast/mul are
                            # deferred one unit so nothing waits on them
                            stg = small.tile([HD + 1, QH], F32, tag="stg", name="stg", bufs=4)
                            nc.vector.tensor_copy(out=stg[:], in_=cxs[u][:])

                            def tail(hp=hp, mo=mo, col0=col0, stg=stg):
                                rec = small.tile([1, QH], F32, tag="rec", name="rec", bufs=2)
                                nc.vector.reciprocal(rec[:], stg[HD : HD + 1, :])
                                bcs = small.tile([HD, QH], F32, tag="bcs", bufs=2)
                                nc.gpsimd.partition_broadcast(bcs[:], rec[0:1, :])
                                if mo == 0:
                                    nc.vector.tensor_mul(
                                        CT[hp][0:HD, col0 : col0 + QH],
                                        stg[0:HD, :],
                                        bcs[:],
                                    )
                                else:
                                    tmp = small.tile([HD, QH], BF16, tag="tmp")
                                    nc.vector.tensor_mul(tmp[:], stg[0:HD, :], bcs[:])
                                    nc.sync.dma_start(
                                        CT[hp][mo : mo + HD, col0 : col0 + QH], tmp[:]
                                    )

                            pending.append(tail)
                        for fn in prev_tails:
                            fn()

                for fn in pending:
                    fn()

            def outproj_units(qq, tp_pool, acc_pool):
                t0 = qq * (QS // P)
                for t in range(t0, t0 + QS // P):
                    for c in range(D // OC):
                        even = (t * (D // OC) + c) % 2 == 0
                        po = (tp_pool if even else acc_pool).tile(
                            [P, OC], F32, tag="tp" if even else "acc", name="po"
                        )
                        for dd in range(MT):
                            nc.tensor.matmul(
                                po[:],
                                lhsT=CT[dd][:, ts(t, P)],
                                rhs=wos[:, dd, ts(c, OC)],
                                start=(dd == 0),
                                stop=(dd == MT - 1),
                            )
                        osb = osb_pool.tile([P, OC], F32, tag="osb")
                        nc.vector.tensor_add(osb[:], po[:], bob[:, ts(c, OC)])
                        nc.sync.dma_start(out[ts(t, P), ts(c, OC)], osb[:])
                        yield

            def outproj(qq, tp_pool, acc_pool):
                for _ in outproj_units(qq, tp_pool, acc_pool):
                    pass

            # ---- phase 1: K.T and V' (full-S prerequisites of attention) ----
            with (
                tc.tile_pool(name="ps1tp", bufs=3, space="PSUM") as ps1tp,
                tc.tile_pool(name="ps1acc", bufs=4, space="PSUM") as ps1acc,
            ):
                for c in range(D // OC):
                    bp = ps1acc.tile([P, OC], F32, tag="acc", name="bp")
                    nc.tensor.matmul(
                        bp[:], lhsT=ones[0:1, 0:P], rhs=bos[0:1, ts(c, OC)],
                        start=True, stop=True,
                    )
                    nc.vector.tensor_copy(out=bob[:, ts(c, OC)], in_=bp[:])
                for c in range(DL // min(OC, DL)):
                    w_ = min(OC, DL)
                    bp = ps1acc.tile([P, w_], F32, tag="acc", name="bp2")
                    nc.tensor.matmul(
                        bp[:], lhsT=ones[0:1, 0:P], rhs=bvs[0:1, ts(c, w_)],
                        start=True, stop=True,
                    )
                    nc.vector.tensor_copy(out=bvb[:, ts(c, w_)], in_=bp[:])
                wks = load_w(wk)
                for nch in range(S // NCH):
                    proj_T(xk, wks, bks, KT, nch, ps1tp, ps1acc)
                wvs = load_w(wv)
                vproj(wvs, ps1tp, ps1acc)

            # ---- phase 2: Q.T chunks, attention, out-proj ----
            with (
                tc.tile_pool(name="ps2tp", bufs=1, space="PSUM") as ps2tp,
                tc.tile_pool(name="ps2acc", bufs=1, space="PSUM") as ps2acc,
                tc.tile_pool(name="ps2sc", bufs=2, space="PSUM") as ps2sc,
                tc.tile_pool(name="ps2cx", bufs=2, space="PSUM") as ps2cx,
            ):
                wqs = load_w(wq)
                CPQ = QS // NCH  # projection chunks per q-superchunk
                from itertools import chain

                for nch in range(CPQ):
                    proj_T(xq, wqs, bqs, QT, nch, ps2tp, ps2acc)
                for qq in range(NQ):
                    if qq + 1 < NQ:
                        filler = chain.from_iterable(
                            proj_units(xq, wqs, bqs, QT, nch, ps2tp, ps2acc)
                            for nch in range((qq + 1) * CPQ, (qq + 2) * CPQ)
                        )
                        n_units = CPQ * (NCH // P + MT)
                    elif qq >= 1:
                        filler = outproj_units(qq - 1, ps2tp, ps2acc)
                        n_units = (QS // P) * (D // OC)
                    else:
                        filler = None
                        n_units = 1
                    attention(
                        qq,
                        ps2sc,
                        ps2cx,
                        ps2tp,
                        filler,
                        pump_every=max(1, (HL * ST) // max(n_units, 1)),
                    )
                    if filler is not None:
                        for _ in filler:
                            pass
                outproj(NQ - 1, ps2tp, ps2acc)
                if NQ == 1:
                    pass
                else:
                    for qq in range(NQ - 2):
                        outproj(qq, ps2tp, ps2acc)

    nc.compile()
    return nc


_NC_CACHE = {}


def _get_nc(S, D, DL, HD):
    key = (S, D, DL, HD)
    if key not in _NC_CACHE:
        _NC_CACHE[key] = build_nc(S, D, DL, HD)
    return _NC_CACHE[key]


def _shard_inputs(q, k, v, mask, Wq, bq, Wk, bk, Wv, bv, Wo, bo):
    q, k, v = np.asarray(q), np.asarray(k), np.asarray(v)
    mask = np.asarray(mask)
    Wq, Wk, Wv, Wo = np.asarray(Wq), np.asarray(Wk), np.asarray(Wv), np.asarray(Wo)
    bq, bk, bv, bo = np.asarray(bq), np.asarray(bk), np.asarray(bv), np.asarray(bo)

    B, S, D = q.shape  # 4, 2048, 1024
    G = 2  # head-groups (tensor-parallel factor); B*G = 8 cores
    DL = D // G
    MT = DL // P
    ST = S // P

    f32 = np.float32
    in_maps = []
    for c in range(B * G):
        b, g = c // G, c % G
        sl = slice(g * DL, (g + 1) * DL)
        bo_core = bo if g == 0 else np.zeros_like(bo)
        in_maps.append(
            {
                "xq": np.ascontiguousarray(q[b], dtype=f32),
                "xk": np.ascontiguousarray(k[b], dtype=f32),
                "xv": np.ascontiguousarray(v[b], dtype=f32),
                "msk": np.ascontiguousarray(
                    mask[b, 0, 0].reshape(ST, P).T, dtype=np.int32
                ),
                "wq": np.ascontiguousarray(Wq[:, sl], dtype=f32),
                "wk": np.ascontiguousarray(Wk[:, sl], dtype=f32),
                "wv": np.ascontiguousarray(Wv[:, sl], dtype=f32),
                "wo": np.ascontiguousarray(Wo[sl, :], dtype=f32),
                "bq": np.ascontiguousarray(bq[sl].reshape(MT, P).T, dtype=f32),
                "bk": np.ascontiguousarray(bk[sl].reshape(MT, P).T, dtype=f32),
                "bv": np.ascontiguousarray(bv[sl].reshape(1, DL), dtype=f32),
                "bo": np.ascontiguousarray(bo_core.reshape(1, D), dtype=f32),
            }
        )
    return in_maps


def kernel(q, k, v, mask, Wq, bq, Wk, bk, Wv, bv, Wo, bo):
    from concourse.bass_utils import run_bass_kernel_spmd

    q = np.asarray(q)
    B, S, D = q.shape  # 4, 2048, 1024
    G = 2
    nc = _get_nc(S, D, D // G, 64)
    in_maps = _shard_inputs(q, k, v, mask, Wq, bq, Wk, bk, Wv, bv, Wo, bo)

    res = run_bass_kernel_spmd(nc, in_maps, core_ids=list(range(B * G)))
    parts = [r["out"] for r in res.results]
    outf = np.stack([parts[b * G] + parts[b * G + 1] for b in range(B)], axis=0)
    return outf.astype(np.float32)



# revision 4
# speedup vs baseline: 1.5717x; 1.5717x over previous
"""Multi-head attention (B=4, S=2048, D=1024, H=16) on 8 trn2 NeuronCores.

Sharding: 8 cores = 4 batches x 2 head-groups. Core c handles batch c//2 and
heads [8g, 8g+8) where g = c%2 (tensor-parallel: Wq/Wk/Wv column-sliced,
Wo row-sliced). Each core returns a partial output [S, D]; the host sums the
two head-group partials per batch.

Key-compaction: the mask drops a key entirely (exp(-1e9) == 0), so the host
gathers only the unmasked key rows of k/v per batch (padded to a 128-multiple
SK; pad slots are masked out on device). For the ~50% random mask this nearly
halves all K-side work (K/V projection, scores, exp, context).

All x / W tensors travel as bf16 (host casts): half the DMA traffic, PE
transposes run at 1 cycle/row instead of 2, and no on-device f32->bf16 casts.

Per-core dataflow (everything stays transposed until the output projection):
  xT tiles (PE transpose, bf16) -> Q.T/K.T = W.T @ X.T, V natural (ones
  column appended) -> scores.T = K @ Q.T -> exp+mask+scale in one ACT op
  -> ctxU.T = V'.T @ expS.T (last row = softmax denominator) -> normalize
  (fast-approx reciprocal) -> out = ctx.T.T @ Wo + bo.
"""

import sys

if "/opt/trn_rl_repo" not in sys.path:
    sys.path.append("/opt/trn_rl_repo")

import numpy as np
import ml_dtypes

import concourse.bass as bass
import concourse.bacc as bacc
import concourse.tile as tile
from concourse import mybir
from concourse.bass import ts
from concourse.masks import make_identity

F32 = mybir.dt.float32
BF16 = mybir.dt.bfloat16
I32 = mybir.dt.int32
EXP = mybir.ActivationFunctionType.Exp

P = 128
NPBF = ml_dtypes.bfloat16


def build_nc(S=2048, D=1024, DL=512, HD=64, SK=1152):
    """Per-core Bass program. DL = local out dim (heads*HD), SK = key len."""
    ST = S // P  # q token tiles
    SKT = SK // P  # key token tiles
    KD = D // P  # contraction tiles over D
    MT = DL // P  # local d-col tiles
    HL = DL // HD  # local heads
    HPT = P // HD  # heads per 128-partition tile (2)
    NCH = min(512, S)  # projection token-chunk
    QS = min(1024, S)  # attention q superchunk (<=2 psum banks)
    QH = min(512, QS)  # one-bank half
    NH = QS // QH
    NQ = S // QS  # q-superchunks
    OC = min(512, D)  # out-proj col chunk
    scale = float(1.0 / (np.sqrt(np.float32(HD)) + 1e-8))

    # K-side token chunks (SK may not be a NCH multiple)
    kchunks = []
    o = 0
    while o < SK:
        w_ = min(NCH, SK - o)
        kchunks.append((o, w_))
        o += w_

    nc = bacc.Bacc("TRN2", target_bir_lowering=False, debug=False)

    xq = nc.dram_tensor("xq", [S, D], BF16, kind="ExternalInput")
    xk = nc.dram_tensor("xk", [SK, D], BF16, kind="ExternalInput")
    xv = nc.dram_tensor("xv", [SK, D], BF16, kind="ExternalInput")
    msk = nc.dram_tensor("msk", [P, SKT], I32, kind="ExternalInput")
    wq = nc.dram_tensor("wq", [D, DL], BF16, kind="ExternalInput")
    wk = nc.dram_tensor("wk", [D, DL], BF16, kind="ExternalInput")
    wv = nc.dram_tensor("wv", [D, DL], BF16, kind="ExternalInput")
    wo = nc.dram_tensor("wo", [DL, D], BF16, kind="ExternalInput")
    bq = nc.dram_tensor("bq", [P, MT], F32, kind="ExternalInput")
    bk = nc.dram_tensor("bk", [P, MT], F32, kind="ExternalInput")
    bv = nc.dram_tensor("bv", [1, DL], F32, kind="ExternalInput")
    bo = nc.dram_tensor("bo", [1, D], F32, kind="ExternalInput")
    out = nc.dram_tensor("out", [S, D], F32, kind="ExternalOutput")

    with tile.TileContext(nc) as tc, nc.allow_low_precision("bf16 operands are rounded by design"):
        with (
            tc.tile_pool(name="pers", bufs=1) as pers,
            tc.tile_pool(name="wpool", bufs=2) as wpool,
            tc.tile_pool(name="xnat", bufs=3) as xnat_pool,
            tc.tile_pool(name="xt", bufs=KD + 1) as xt_pool,
            tc.tile_pool(name="exp", bufs=10) as ex_pool,
            tc.tile_pool(name="osb", bufs=3) as osb_pool,
            tc.tile_pool(name="small", bufs=2) as small,
        ):
            # ---- constants ----
            ident = pers.tile([P, P], BF16, tag="ident")
            make_identity(nc, ident[:])

            mi = pers.tile([P, SKT], I32, tag="mi")
            nc.sync.dma_start(mi[:], msk[:, :])
            mf = pers.tile([P, SKT], F32, tag="mf")
            nc.vector.tensor_copy(out=mf[:], in_=mi[:])
            mb = pers.tile([P, SKT], F32, tag="mb")
            nc.vector.tensor_scalar_mul(mb[:], mf[:], -1.0e9)

            bqs = pers.tile([P, MT], F32, tag="bqs")
            nc.sync.dma_start(bqs[:], bq[:, :])
            bks = pers.tile([P, MT], F32, tag="bks")
            nc.sync.dma_start(bks[:], bk[:, :])
            bvs = pers.tile([1, DL], F32, tag="bvs")
            nc.sync.dma_start(bvs[:], bv[:, :])
            bos = pers.tile([1, D], F32, tag="bos")
            nc.sync.dma_start(bos[:], bo[:, :])
            bvb = pers.tile([P, DL], F32, tag="bvb")
            nc.gpsimd.partition_broadcast(bvb[:], bvs[0:1, :])
            bob = pers.tile([P, D], F32, tag="bob")
            nc.gpsimd.partition_broadcast(bob[:], bos[0:1, :])

            # persistent activation stores
            KT = [pers.tile([P, SK], BF16, tag=f"kt{m}", name=f"kt{m}") for m in range(MT)]
            QT = [pers.tile([P, S], BF16, tag=f"qt{m}", name=f"qt{m}") for m in range(MT)]
            CT = [pers.tile([P, S], BF16, tag=f"ct{m}", name=f"ct{m}") for m in range(MT)]
            VP = [pers.tile([P, HL * (HD + 1)], BF16, tag=f"vp{t}", name=f"vp{t}") for t in range(SKT)]
            for t in range(SKT):
                nc.gpsimd.memset(VP[t][:], 1.0)

            def load_w(wdram):
                w = wpool.tile([P, KD, DL], BF16, tag="w", name="w")
                nc.sync.dma_start(w[:], wdram.rearrange("(k p) n -> p k n", p=P))
                return w

            wos = pers.tile([P, MT, D], BF16, tag="wos")

            def tpose(dst, src, tp_slot):
                """dst[128, 128] (SBUF) = src[128, 128].T via PE (bf16)."""
                nc.tensor.transpose(tp_slot, src, ident[:])
                nc.vector.tensor_copy(out=dst, in_=tp_slot)

            def proj_units(xdram, wsb, bias_sb, dst_tiles, tok0, ntok, tp_pool, acc_pool):
                """dst[m][:, tok0:tok0+ntok] = (x @ w + b).T; yields at unit edges."""
                nt = ntok // P
                xts = [
                    xt_pool.tile([P, NCH], BF16, tag="xt", name="xt") for _ in range(KD)
                ]
                for i in range(nt):
                    xn = xnat_pool.tile([P, D], BF16, tag="xnat")
                    nc.sync.dma_start(xn[:], xdram[tok0 + i * P : tok0 + (i + 1) * P, :])
                    tp4 = (tp_pool if i % 2 == 0 else acc_pool).tile(
                        [P, 4, P], BF16, tag="tp" if i % 2 == 0 else "acc", name="tp4"
                    )
                    for kk in range(KD):
                        tpose(xts[kk][:, ts(i, P)], xn[:, ts(kk, P)], tp4[:, kk % 4, :])
                    yield
                for m in range(MT):
                    acc = acc_pool.tile([P, NCH], F32, tag="acc")
                    for kk in range(KD):
                        nc.tensor.matmul(
                            acc[:, 0:ntok],
                            lhsT=wsb[:, kk, ts(m, P)],
                            rhs=xts[kk][:, 0:ntok],
                            start=(kk == 0),
                            stop=(kk == KD - 1),
                        )
                    nc.vector.tensor_scalar_add(
                        dst_tiles[m][:, tok0 : tok0 + ntok], acc[:, 0:ntok], bias_sb[:, m : m + 1]
                    )
                    yield

            def proj_T(xdram, wsb, bias_sb, dst_tiles, tok0, ntok, tp_pool, acc_pool):
                for _ in proj_units(
                    xdram, wsb, bias_sb, dst_tiles, tok0, ntok, tp_pool, acc_pool
                ):
                    pass

            def vproj(wsb, tp_pool, acc_pool):
                """VP[t][:, h*(HD+1):+HD] = (xv @ wv + bv)[t-tile, h-slice]."""
                for t in range(SKT):
                    xn = xnat_pool.tile([P, D], BF16, tag="xnat")
                    nc.sync.dma_start(xn[:], xv[ts(t, P), :])
                    xts = []
                    tp4 = tp_pool.tile([P, 4, P], BF16, tag="tp", name="tp4")
                    for kk in range(KD):
                        xt = xt_pool.tile([P, NCH], BF16, tag="xt", name="xt")
                        tpose(xt[:, 0:P], xn[:, ts(kk, P)], tp4[:, kk % 4, :])
                        xts.append(xt)
                    acc = acc_pool.tile([P, DL], F32, tag="acc")
                    for kk in range(KD):
                        nc.tensor.matmul(
                            acc[:],
                            lhsT=xts[kk][:, 0:P],
                            rhs=wsb[:, kk, :],
                            start=(kk == 0),
                            stop=(kk == KD - 1),
                        )
                    for h in range(HL):
                        nc.vector.tensor_add(
                            VP[t][:, h * (HD + 1) : h * (HD + 1) + HD],
                            acc[:, ts(h, HD)],
                            bvb[:, ts(h, HD)],
                        )

            def attention(qq, sc_pool, cx_pool, filler=None, pump_every=8):
                it = 0
                pending = []  # deferred normalize work (recip/broadcast/mul)
                for hp in range(HL // HPT):  # head pairs share a KT/QT tile
                    for q5 in range(NH):
                        col0 = qq * QS + q5 * QH
                        cxs = [
                            cx_pool.tile([HD + 1, QH], F32, tag="cx", name="cx")
                            for _ in range(HPT)
                        ]
                        for kt in range(SKT):
                            # one PSUM supertile holds both heads' score chunk;
                            # the two K=64 matmuls run concurrently (row groups
                            # 0-63 / 64-127), one ACT exp covers both
                            sc = sc_pool.tile([P, HPT * QH], F32, tag="sc")
                            for u in range(HPT):
                                mo = u * HD
                                nc.tensor.matmul(
                                    sc[:, ts(u, QH)],
                                    lhsT=KT[hp][mo : mo + HD, ts(kt, P)],
                                    rhs=QT[hp][mo : mo + HD, col0 : col0 + QH],
                                    start=True,
                                    stop=True,
                                )
                            ex = ex_pool.tile([P, HPT * QH], BF16, tag="ex")
                            nc.scalar.activation(
                                ex[:], sc[:], EXP, bias=mb[:, kt : kt + 1], scale=scale
                            )
                            for u in range(HPT):
                                h = hp * HPT + u
                                nc.tensor.matmul(
                                    cxs[u][:],
                                    lhsT=VP[kt][:, h * (HD + 1) : (h + 1) * (HD + 1)],
                                    rhs=ex[:, ts(u, QH)],
                                    start=(kt == 0),
                                    stop=(kt == SKT - 1),
                                )
                            it += 1
                            if filler is not None and it % pump_every == 0:
                                next(filler, None)
                        prev_tails = pending
                        pending = []
                        for u in range(HPT):
                            mo = u * HD
                            # the cheap DVE copy (emitted now, ahead of the
                            # previous unit's reciprocals in DVE order) frees
                            # the ctx PSUM slot; recip/broadcast/mul are
                            # deferred one unit so nothing waits on them
                            stg = small.tile([HD + 1, QH], F32, tag="stg", name="stg", bufs=4)
                            nc.vector.tensor_copy(out=stg[:], in_=cxs[u][:])

                            def tail(hp=hp, mo=mo, col0=col0, stg=stg):
                                rec = small.tile([1, QH], F32, tag="rec", name="rec", bufs=2)
                                nc.vector.reciprocal(rec[:], stg[HD : HD + 1, :])
                                bcs = small.tile([HD, QH], F32, tag="bcs", bufs=2)
                                nc.gpsimd.partition_broadcast(bcs[:], rec[0:1, :])
                                if mo == 0:
                                    nc.vector.tensor_mul(
                                        CT[hp][0:HD, col0 : col0 + QH],
                                        stg[0:HD, :],
                                        bcs[:],
                                    )
                                else:
                                    tmp = small.tile([HD, QH], BF16, tag="tmp")
                                    nc.vector.tensor_mul(tmp[:], stg[0:HD, :], bcs[:])
                                    nc.sync.dma_start(
                                        CT[hp][mo : mo + HD, col0 : col0 + QH], tmp[:]
                                    )

                            pending.append(tail)
                        for fn in prev_tails:
                            fn()

                for fn in pending:
                    fn()

            def outproj_units(qq, pool_a, pool_b):
                t0 = qq * (QS // P)
                for t in range(t0, t0 + QS // P):
                    for c in range(D // OC):
                        even = (t * (D // OC) + c) % 2 == 0
                        po = (pool_a if even else pool_b).tile(
                            [P, OC], F32, tag=("tp" if pool_a is not pool_b else "acc") if even else "acc", name="po"
                        )
                        for dd in range(MT):
                            nc.tensor.matmul(
                                po[:],
                                lhsT=CT[dd][:, ts(t, P)],
                                rhs=wos[:, dd, ts(c, OC)],
                                start=(dd == 0),
                                stop=(dd == MT - 1),
                            )
                        osb = osb_pool.tile([P, OC], F32, tag="osb")
                        nc.vector.tensor_add(osb[:], po[:], bob[:, ts(c, OC)])
                        nc.sync.dma_start(out[ts(t, P), ts(c, OC)], osb[:])
                        yield

            # ---- phase 1: K.T and V' (full-S prerequisites of attention) ----
            with (
                tc.tile_pool(name="ps1tp", bufs=3, space="PSUM") as ps1tp,
                tc.tile_pool(name="ps1acc", bufs=4, space="PSUM") as ps1acc,
            ):
                wks = load_w(wk)
                for tok0, ntok in kchunks:
                    proj_T(xk, wks, bks, KT, tok0, ntok, ps1tp, ps1acc)
                wvs = load_w(wv)
                vproj(wvs, ps1tp, ps1acc)

            # ---- phase 2: Q.T chunks, attention, out-proj ----
            with (
                tc.tile_pool(name="ps2tp", bufs=1, space="PSUM") as ps2tp,
                tc.tile_pool(name="ps2acc", bufs=1, space="PSUM") as ps2acc,
                tc.tile_pool(name="ps2sc", bufs=2, space="PSUM") as ps2sc,
                tc.tile_pool(name="ps2cx", bufs=2, space="PSUM") as ps2cx,
            ):
                wqs = load_w(wq)
                nc.sync.dma_start(wos[:], wo.rearrange("(m p) n -> p m n", p=P))
                CPQ = QS // NCH  # projection chunks per q-superchunk
                from itertools import chain

                n_att_its = (HL // HPT) * NH * SKT
                for nch in range(CPQ):
                    proj_T(xq, wqs, bqs, QT, nch * NCH, NCH, ps2tp, ps2acc)
                for qq in range(NQ):
                    if qq + 1 < NQ:
                        filler = chain.from_iterable(
                            proj_units(xq, wqs, bqs, QT, nch * NCH, NCH, ps2tp, ps2acc)
                            for nch in range((qq + 1) * CPQ, (qq + 2) * CPQ)
                        )
                        n_units = CPQ * (NCH // P + MT)
                    elif qq >= 1:
                        filler = outproj_units(qq - 1, ps2tp, ps2acc)
                        n_units = (QS // P) * (D // OC)
                    else:
                        filler = None
                        n_units = 1
                    attention(
                        qq,
                        ps2sc,
                        ps2cx,
                        filler,
                        pump_every=max(1, n_att_its // max(n_units, 1)),
                    )
                    if filler is not None:
                        for _ in filler:
                            pass
                if NQ >= 2:
                    for qq in range(NQ - 2):
                        outproj_units_done = outproj_units(qq, ps2tp, ps2acc)
                        for _ in outproj_units_done:
                            pass

            # ---- phase 3: final out-proj with deep PSUM so the epilogue
            # (vector add + store) trails by less than a unit ----
            with tc.tile_pool(name="ps3", bufs=4, space="PSUM") as ps3:
                for _ in outproj_units(NQ - 1, ps3, ps3):
                    pass

    nc.compile()
    return nc


_NC_CACHE = {}


def _get_nc(S, D, DL, HD, SK):
    key = (S, D, DL, HD, SK)
    if key not in _NC_CACHE:
        _NC_CACHE[key] = build_nc(S, D, DL, HD, SK)
    return _NC_CACHE[key]


def _shard_inputs(q, k, v, mask, Wq, bq, Wk, bk, Wv, bv, Wo, bo):
    q, k, v = np.asarray(q), np.asarray(k), np.asarray(v)
    mask = np.asarray(mask)
    Wq, Wk, Wv, Wo = np.asarray(Wq), np.asarray(Wk), np.asarray(Wv), np.asarray(Wo)
    bq, bk, bv, bo = np.asarray(bq), np.asarray(bk), np.asarray(bv), np.asarray(bo)

    B, S, D = q.shape  # 4, 2048, 1024
    G = 2  # head-groups (tensor-parallel factor); B*G = 8 cores
    DL = D // G
    MT = DL // P

    # key compaction: gather unmasked key rows, pad to a 128-multiple
    keep = [np.flatnonzero(mask[b, 0, 0] == 0) for b in range(B)]
    maxc = max(max((len(ix) for ix in keep), default=1), 1)
    SK = min(S, ((maxc + P - 1) // P) * P)
    SKT = SK // P

    f32 = np.float32
    xk_c, xv_c, msk_c = [], [], []
    for b in range(B):
        if SK == S:
            # fallback: no compaction, original order + original mask
            xk_c.append(np.ascontiguousarray(k[b], dtype=NPBF))
            xv_c.append(np.ascontiguousarray(v[b], dtype=NPBF))
            msk_c.append(
                np.ascontiguousarray(mask[b, 0, 0].reshape(SKT, P).T, dtype=np.int32)
            )
        else:
            ix = keep[b][:SK]
            n = len(ix)
            kb = np.zeros((SK, D), dtype=NPBF)
            vb = np.zeros((SK, D), dtype=NPBF)
            kb[:n] = k[b][ix].astype(NPBF)
            vb[:n] = v[b][ix].astype(NPBF)
            mb_ = np.zeros((SK,), dtype=np.int32)
            mb_[n:] = 1
            xk_c.append(kb)
            xv_c.append(vb)
            msk_c.append(np.ascontiguousarray(mb_.reshape(SKT, P).T, dtype=np.int32))

    xq_b = [np.ascontiguousarray(q[b], dtype=NPBF) for b in range(B)]

    in_maps = []
    for c in range(B * G):
        b, g = c // G, c % G
        sl = slice(g * DL, (g + 1) * DL)
        bo_core = bo if g == 0 else np.zeros_like(bo)
        in_maps.append(
            {
                "xq": xq_b[b],
                "xk": xk_c[b],
                "xv": xv_c[b],
                "msk": msk_c[b],
                "wq": np.ascontiguousarray(Wq[:, sl].astype(NPBF)),
                "wk": np.ascontiguousarray(Wk[:, sl].astype(NPBF)),
                "wv": np.ascontiguousarray(Wv[:, sl].astype(NPBF)),
                "wo": np.ascontiguousarray(Wo[sl, :].astype(NPBF)),
                "bq": np.ascontiguousarray(bq[sl].reshape(MT, P).T, dtype=f32),
                "bk": np.ascontiguousarray(bk[sl].reshape(MT, P).T, dtype=f32),
                "bv": np.ascontiguousarray(bv[sl].reshape(1, DL), dtype=f32),
                "bo": np.ascontiguousarray(bo_core.reshape(1, D), dtype=f32),
            }
        )
    return in_maps, SK


def kernel(q, k, v, mask, Wq, bq, Wk, bk, Wv, bv, Wo, bo):
    from concourse.bass_utils import run_bass_kernel_spmd

    q = np.asarray(q)
    B, S, D = q.shape  # 4, 2048, 1024
    G = 2
    in_maps, SK = _shard_inputs(q, k, v, mask, Wq, bq, Wk, bk, Wv, bv, Wo, bo)
    nc = _get_nc(S, D, D // G, 64, SK)

    res = run_bass_kernel_spmd(nc, in_maps, core_ids=list(range(B * G)))
    parts = [r["out"] for r in res.results]
    outf = np.stack([parts[b * G] + parts[b * G + 1] for b in range(B)], axis=0)
    return outf.astype(np.float32)


# revision 7
# speedup vs baseline: 1.9149x; 1.2184x over previous
"""Multi-head attention (B=4, S=2048, D=1024, H=16) on 8 trn2 NeuronCores.

Sharding: 8 cores = 4 batches x 2 head-groups. Core c handles batch c//2 and
heads [8g, 8g+8) where g = c%2 (tensor-parallel: Wq/Wk/Wv column-sliced,
Wo row-sliced). Each core returns a partial output [S, D]; the host sums the
two head-group partials per batch.

Key-compaction: the mask drops a key entirely (exp(-1e9) == 0), so the host
gathers only the unmasked key rows of k/v per batch (padded to a 128-multiple
SK; pad slots are masked out on device). For the ~50% random mask this nearly
halves all K-side work (K/V projection, scores, exp, context).

All x / W tensors travel as bf16 (host casts): half the DMA traffic, PE
transposes run at 1 cycle/row instead of 2, and no on-device f32->bf16 casts.

Per-core dataflow (everything stays transposed until the output projection):
  xT tiles (PE transpose, bf16) -> Q.T/K.T = W.T @ X.T, V natural (ones
  column appended) -> scores.T = K @ Q.T -> exp+mask+scale in one ACT op
  -> ctxU.T = V'.T @ expS.T (last row = softmax denominator) -> normalize
  (fast-approx reciprocal) -> out = ctx.T.T @ Wo + bo.
"""

import sys

if "/opt/trn_rl_repo" not in sys.path:
    sys.path.append("/opt/trn_rl_repo")

import numpy as np
import ml_dtypes

import concourse.bass as bass
import concourse.bacc as bacc
import concourse.tile as tile
from concourse import mybir
from concourse.bass import ts
from concourse.masks import make_identity

F32 = mybir.dt.float32
BF16 = mybir.dt.bfloat16
I32 = mybir.dt.int32
EXP = mybir.ActivationFunctionType.Exp

P = 128
NPBF = ml_dtypes.bfloat16


def build_nc(S=2048, D=1024, DL=512, HD=64, SK=1152):
    """Per-core Bass program. DL = local out dim (heads*HD), SK = key len."""
    ST = S // P  # q token tiles
    SKT = SK // P  # key token tiles
    KD = D // P  # contraction tiles over D
    MT = DL // P  # local d-col tiles
    HL = DL // HD  # local heads
    HPT = P // HD  # heads per 128-partition tile (2)
    NCH = min(512, S)  # projection token-chunk
    QS = min(1024, S)  # attention q superchunk (<=2 psum banks)
    QH = min(512, QS)  # one-bank half
    NH = QS // QH
    NQ = S // QS  # q-superchunks
    OC = min(512, D)  # out-proj col chunk
    scale = float(1.0 / (np.sqrt(np.float32(HD)) + 1e-8))

    # K-side token chunks (SK may not be a NCH multiple)
    kchunks = []
    o = 0
    while o < SK:
        w_ = min(NCH, SK - o)
        kchunks.append((o, w_))
        o += w_

    nc = bacc.Bacc("TRN2", target_bir_lowering=False, debug=False)

    xq = nc.dram_tensor("xq", [S, D], BF16, kind="ExternalInput")
    xk = nc.dram_tensor("xk", [SK, D], BF16, kind="ExternalInput")
    xv = nc.dram_tensor("xv", [SK, D], BF16, kind="ExternalInput")
    msk = nc.dram_tensor("msk", [P, SKT], I32, kind="ExternalInput")
    wq = nc.dram_tensor("wq", [D, DL], BF16, kind="ExternalInput")
    wk = nc.dram_tensor("wk", [D, DL], BF16, kind="ExternalInput")
    wv = nc.dram_tensor("wv", [D, DL], BF16, kind="ExternalInput")
    wo = nc.dram_tensor("wo", [DL, D], BF16, kind="ExternalInput")
    bq = nc.dram_tensor("bq", [P, MT], F32, kind="ExternalInput")
    bk = nc.dram_tensor("bk", [P, MT], F32, kind="ExternalInput")
    bv = nc.dram_tensor("bv", [1, DL], F32, kind="ExternalInput")
    bo = nc.dram_tensor("bo", [1, D], F32, kind="ExternalInput")
    out = nc.dram_tensor("out", [S, D], F32, kind="ExternalOutput")

    with tile.TileContext(nc) as tc, nc.allow_low_precision("bf16 operands are rounded by design"):
        with (
            tc.tile_pool(name="pers", bufs=1) as pers,
            tc.tile_pool(name="wpool", bufs=2) as wpool,
            tc.tile_pool(name="xnat", bufs=3) as xnat_pool,
            tc.tile_pool(name="xt", bufs=KD + 1) as xt_pool,
            tc.tile_pool(name="exp", bufs=10) as ex_pool,
            tc.tile_pool(name="osb", bufs=3) as osb_pool,
            tc.tile_pool(name="small", bufs=2) as small,
        ):
            # ---- constants (only what the first K-proj chunk needs; the
            # rest is emitted after it so the first DMAs/gpsimd ops on the
            # queues are the ones the critical path waits on) ----
            ident = pers.tile([P, P], BF16, tag="ident")
            make_identity(nc, ident[:])
            bks = pers.tile([P, MT], F32, tag="bks")
            nc.sync.dma_start(bks[:], bk[:, :])

            mi = pers.tile([P, SKT], I32, tag="mi")
            mf = pers.tile([P, SKT], F32, tag="mf")
            mb = pers.tile([P, SKT], F32, tag="mb")
            bqs = pers.tile([P, MT], F32, tag="bqs")
            bvs = pers.tile([1, DL], F32, tag="bvs")
            bos = pers.tile([1, D], F32, tag="bos")
            bvb = pers.tile([P, DL], F32, tag="bvb")
            bob = pers.tile([P, D], F32, tag="bob")

            def late_consts():
                nc.sync.dma_start(mi[:], msk[:, :])
                nc.vector.tensor_copy(out=mf[:], in_=mi[:])
                nc.vector.tensor_scalar_mul(mb[:], mf[:], -1.0e9)
                nc.sync.dma_start(bqs[:], bq[:, :])
                nc.sync.dma_start(bvs[:], bv[:, :])
                nc.sync.dma_start(bos[:], bo[:, :])
                nc.gpsimd.partition_broadcast(bvb[:], bvs[0:1, :])
                nc.gpsimd.partition_broadcast(bob[:], bos[0:1, :])
                for t in range(SKT):
                    nc.gpsimd.memset(VP[t][:], 1.0)

            # persistent activation stores
            KT = [pers.tile([P, SK], BF16, tag=f"kt{m}", name=f"kt{m}") for m in range(MT)]
            QT = [pers.tile([P, S], BF16, tag=f"qt{m}", name=f"qt{m}") for m in range(MT)]
            CT = [pers.tile([P, S], BF16, tag=f"ct{m}", name=f"ct{m}") for m in range(MT)]
            VP = [pers.tile([P, HL * (HD + 1)], BF16, tag=f"vp{t}", name=f"vp{t}") for t in range(SKT)]

            def load_w(wdram):
                w = wpool.tile([P, KD, DL], BF16, tag="w", name="w")
                nc.sync.dma_start(w[:], wdram.rearrange("(k p) n -> p k n", p=P))
                return w

            wos = pers.tile([P, MT, D], BF16, tag="wos")

            def tpose(dst, src, tp_slot):
                """dst[128, 128] (SBUF) = src[128, 128].T via PE (bf16)."""
                nc.tensor.transpose(tp_slot, src, ident[:])
                nc.vector.tensor_copy(out=dst, in_=tp_slot)

            def proj_units(xdram, wsb, bias_sb, dst_tiles, tok0, ntok, tp_pool, acc_pool):
                """dst[m][:, tok0:tok0+ntok] = (x @ w + b).T; yields at unit edges."""
                nt = ntok // P
                xts = [
                    xt_pool.tile([P, NCH], BF16, tag="xt", name="xt") for _ in range(KD)
                ]
                for i in range(nt):
                    xn = xnat_pool.tile([P, D], BF16, tag="xnat")
                    nc.sync.dma_start(xn[:], xdram[tok0 + i * P : tok0 + (i + 1) * P, :])
                    tp4 = (tp_pool if i % 2 == 0 else acc_pool).tile(
                        [P, 4, P], BF16, tag="tp" if i % 2 == 0 else "acc", name="tp4"
                    )
                    for kk in range(KD):
                        tpose(xts[kk][:, ts(i, P)], xn[:, ts(kk, P)], tp4[:, kk % 4, :])
                    yield
                for m in range(MT):
                    acc = acc_pool.tile([P, NCH], F32, tag="acc")
                    for kk in range(KD):
                        nc.tensor.matmul(
                            acc[:, 0:ntok],
                            lhsT=wsb[:, kk, ts(m, P)],
                            rhs=xts[kk][:, 0:ntok],
                            start=(kk == 0),
                            stop=(kk == KD - 1),
                        )
                    nc.vector.tensor_scalar_add(
                        dst_tiles[m][:, tok0 : tok0 + ntok], acc[:, 0:ntok], bias_sb[:, m : m + 1]
                    )
                    yield

            def proj_T(xdram, wsb, bias_sb, dst_tiles, tok0, ntok, tp_pool, acc_pool):
                for _ in proj_units(
                    xdram, wsb, bias_sb, dst_tiles, tok0, ntok, tp_pool, acc_pool
                ):
                    pass

            def vproj(wsb, tp_pool, acc_pool):
                """VP[t][:, h*(HD+1):+HD] = (xv @ wv + bv)[t-tile, h-slice]."""
                for t in range(SKT):
                    xn = xnat_pool.tile([P, D], BF16, tag="xnat")
                    nc.sync.dma_start(xn[:], xv[ts(t, P), :])
                    xts = []
                    tp4 = tp_pool.tile([P, 4, P], BF16, tag="tp", name="tp4")
                    for kk in range(KD):
                        xt = xt_pool.tile([P, NCH], BF16, tag="xt", name="xt")
                        tpose(xt[:, 0:P], xn[:, ts(kk, P)], tp4[:, kk % 4, :])
                        xts.append(xt)
                    acc = acc_pool.tile([P, DL], F32, tag="acc")
                    for kk in range(KD):
                        nc.tensor.matmul(
                            acc[:],
                            lhsT=xts[kk][:, 0:P],
                            rhs=wsb[:, kk, :],
                            start=(kk == 0),
                            stop=(kk == KD - 1),
                        )
                    for h in range(HL):
                        nc.vector.tensor_add(
                            VP[t][:, h * (HD + 1) : h * (HD + 1) + HD],
                            acc[:, ts(h, HD)],
                            bvb[:, ts(h, HD)],
                        )

            def attention(qq, sc_pool, cx_pool, filler=None, pump_every=8):
                it = 0
                pending = []  # deferred normalize work (recip/broadcast/mul)
                for hp in range(HL // HPT):  # head pairs share a KT/QT tile
                    for q5 in range(NH):
                        col0 = qq * QS + q5 * QH
                        cxs = [
                            cx_pool.tile([HD + 1, QH], F32, tag="cx", name="cx")
                            for _ in range(HPT)
                        ]
                        for kt in range(SKT):
                            # one PSUM supertile holds both heads' score chunk;
                            # the two K=64 matmuls run concurrently (row groups
                            # 0-63 / 64-127), one ACT exp covers both
                            sc = sc_pool.tile([P, HPT * QH], F32, tag="sc")
                            for u in range(HPT):
                                mo = u * HD
                                nc.tensor.matmul(
                                    sc[:, ts(u, QH)],
                                    lhsT=KT[hp][mo : mo + HD, ts(kt, P)],
                                    rhs=QT[hp][mo : mo + HD, col0 : col0 + QH],
                                    start=True,
                                    stop=True,
                                )
                            ex = ex_pool.tile([P, HPT * QH], BF16, tag="ex")
                            nc.scalar.activation(
                                ex[:], sc[:], EXP, bias=mb[:, kt : kt + 1], scale=scale
                            )
                            for u in range(HPT):
                                h = hp * HPT + u
                                nc.tensor.matmul(
                                    cxs[u][:],
                                    lhsT=VP[kt][:, h * (HD + 1) : (h + 1) * (HD + 1)],
                                    rhs=ex[:, ts(u, QH)],
                                    start=(kt == 0),
                                    stop=(kt == SKT - 1),
                                )
                            it += 1
                            if filler is not None and it % pump_every == 0:
                                next(filler, None)
                        prev_tails = pending
                        pending = []
                        for u in range(HPT):
                            mo = u * HD
                            # the cheap DVE copy (emitted now, ahead of the
                            # previous unit's reciprocals in DVE order) frees
                            # the ctx PSUM slot; recip/broadcast/mul are
                            # deferred one unit so nothing waits on them
                            stg = small.tile([HD + 1, QH], F32, tag="stg", name="stg", bufs=4)
                            nc.vector.tensor_copy(out=stg[:], in_=cxs[u][:])

                            def tail(hp=hp, mo=mo, col0=col0, stg=stg):
                                # denominator row lives at partition HD; the
                                # custom gpsimd/DVE ops read absolute partition
                                # 0 on HW, so DMA it down to a base-0 tile first
                                rec = small.tile([1, QH], F32, tag="rec", name="rec", bufs=2)
                                nc.sync.dma_start(rec[0:1, :], stg[HD : HD + 1, :])
                                dnb = small.tile([HD, QH], F32, tag="dnb", name="dnb", bufs=2)
                                nc.gpsimd.partition_broadcast(dnb[:], rec[0:1, :])
                                bcs = small.tile([HD, QH], F32, tag="bcs", bufs=2)
                                nc.vector.reciprocal_approx_fast(bcs[:], dnb[:])
                                if mo == 0:
                                    nc.vector.tensor_mul(
                                        CT[hp][0:HD, col0 : col0 + QH],
                                        stg[0:HD, :],
                                        bcs[:],
                                    )
                                else:
                                    tmp = small.tile([HD, QH], BF16, tag="tmp")
                                    nc.vector.tensor_mul(tmp[:], stg[0:HD, :], bcs[:])
                                    nc.sync.dma_start(
                                        CT[hp][mo : mo + HD, col0 : col0 + QH], tmp[:]
                                    )

                            pending.append(tail)
                        for fn in prev_tails:
                            fn()

                for fn in pending:
                    fn()

            def outproj_units(qq, pool_a, pool_b):
                t0 = qq * (QS // P)
                for t in range(t0, t0 + QS // P):
                    for c in range(D // OC):
                        even = (t * (D // OC) + c) % 2 == 0
                        po = (pool_a if even else pool_b).tile(
                            [P, OC], F32, tag=("tp" if pool_a is not pool_b else "acc") if even else "acc", name="po"
                        )
                        for dd in range(MT):
                            nc.tensor.matmul(
                                po[:],
                                lhsT=CT[dd][:, ts(t, P)],
                                rhs=wos[:, dd, ts(c, OC)],
                                start=(dd == 0),
                                stop=(dd == MT - 1),
                            )
                        osb = osb_pool.tile([P, OC], F32, tag="osb")
                        nc.vector.tensor_add(osb[:], po[:], bob[:, ts(c, OC)])
                        nc.sync.dma_start(out[ts(t, P), ts(c, OC)], osb[:])
                        yield

            # ---- phase 1: K.T and V' (full-S prerequisites of attention) ----
            with (
                tc.tile_pool(name="ps1tp", bufs=3, space="PSUM") as ps1tp,
                tc.tile_pool(name="ps1acc", bufs=4, space="PSUM") as ps1acc,
            ):
                wks = load_w(wk)
                for ci, (tok0, ntok) in enumerate(kchunks):
                    proj_T(xk, wks, bks, KT, tok0, ntok, ps1tp, ps1acc)
                    if ci == 0:
                        late_consts()
                wvs = load_w(wv)
                vproj(wvs, ps1tp, ps1acc)

            # ---- phase 2: Q.T chunks, attention, out-proj ----
            with (
                tc.tile_pool(name="ps2tp", bufs=1, space="PSUM") as ps2tp,
                tc.tile_pool(name="ps2acc", bufs=1, space="PSUM") as ps2acc,
                tc.tile_pool(name="ps2sc", bufs=2, space="PSUM") as ps2sc,
                tc.tile_pool(name="ps2cx", bufs=2, space="PSUM") as ps2cx,
            ):
                wqs = load_w(wq)
                nc.sync.dma_start(wos[:], wo.rearrange("(m p) n -> p m n", p=P))
                CPQ = QS // NCH  # projection chunks per q-superchunk
                from itertools import chain

                n_att_its = (HL // HPT) * NH * SKT
                for nch in range(CPQ):
                    proj_T(xq, wqs, bqs, QT, nch * NCH, NCH, ps2tp, ps2acc)
                for qq in range(NQ):
                    if qq + 1 < NQ:
                        filler = chain.from_iterable(
                            proj_units(xq, wqs, bqs, QT, nch * NCH, NCH, ps2tp, ps2acc)
                            for nch in range((qq + 1) * CPQ, (qq + 2) * CPQ)
                        )
                        n_units = CPQ * (NCH // P + MT)
                    elif qq >= 1:
                        filler = outproj_units(qq - 1, ps2tp, ps2acc)
                        n_units = (QS // P) * (D // OC)
                    else:
                        filler = None
                        n_units = 1
                    attention(
                        qq,
                        ps2sc,
                        ps2cx,
                        filler,
                        pump_every=max(1, n_att_its // max(n_units, 1)),
                    )
                    if filler is not None:
                        for _ in filler:
                            pass
                if NQ >= 2:
                    for qq in range(NQ - 2):
                        outproj_units_done = outproj_units(qq, ps2tp, ps2acc)
                        for _ in outproj_units_done:
                            pass

            # ---- phase 3: final out-proj with deep PSUM so the epilogue
            # (vector add + store) trails by less than a unit ----
            with tc.tile_pool(name="ps3", bufs=4, space="PSUM") as ps3:
                for _ in outproj_units(NQ - 1, ps3, ps3):
                    pass

    nc.compile()
    return nc


_NC_CACHE = {}


def _get_nc(S, D, DL, HD, SK):
    key = (S, D, DL, HD, SK)
    if key not in _NC_CACHE:
        _NC_CACHE[key] = build_nc(S, D, DL, HD, SK)
    return _NC_CACHE[key]


def _shard_inputs(q, k, v, mask, Wq, bq, Wk, bk, Wv, bv, Wo, bo):
    q, k, v = np.asarray(q), np.asarray(k), np.asarray(v)
    mask = np.asarray(mask)
    Wq, Wk, Wv, Wo = np.asarray(Wq), np.asarray(Wk), np.asarray(Wv), np.asarray(Wo)
    bq, bk, bv, bo = np.asarray(bq), np.asarray(bk), np.asarray(bv), np.asarray(bo)

    B, S, D = q.shape  # 4, 2048, 1024
    G = 2  # head-groups (tensor-parallel factor); B*G = 8 cores
    DL = D // G
    MT = DL // P

    # key compaction: gather unmasked key rows, pad to a 128-multiple
    keep = [np.flatnonzero(mask[b, 0, 0] == 0) for b in range(B)]
    maxc = max(max((len(ix) for ix in keep), default=1), 1)
    SK = min(S, ((maxc + P - 1) // P) * P)
    SKT = SK // P

    f32 = np.float32
    xk_c, xv_c, msk_c = [], [], []
    for b in range(B):
        if SK == S:
            # fallback: no compaction, original order + original mask
            xk_c.append(np.ascontiguousarray(k[b], dtype=NPBF))
            xv_c.append(np.ascontiguousarray(v[b], dtype=NPBF))
            msk_c.append(
                np.ascontiguousarray(mask[b, 0, 0].reshape(SKT, P).T, dtype=np.int32)
            )
        else:
            ix = keep[b][:SK]
            n = len(ix)
            kb = np.zeros((SK, D), dtype=NPBF)
            vb = np.zeros((SK, D), dtype=NPBF)
            kb[:n] = k[b][ix].astype(NPBF)
            vb[:n] = v[b][ix].astype(NPBF)
            mb_ = np.zeros((SK,), dtype=np.int32)
            mb_[n:] = 1
            xk_c.append(kb)
            xv_c.append(vb)
            msk_c.append(np.ascontiguousarray(mb_.reshape(SKT, P).T, dtype=np.int32))

    xq_b = [np.ascontiguousarray(q[b], dtype=NPBF) for b in range(B)]

    in_maps = []
    for c in range(B * G):
        b, g = c // G, c % G
        sl = slice(g * DL, (g + 1) * DL)
        bo_core = bo if g == 0 else np.zeros_like(bo)
        in_maps.append(
            {
                "xq": xq_b[b],
                "xk": xk_c[b],
                "xv": xv_c[b],
                "msk": msk_c[b],
                "wq": np.ascontiguousarray(Wq[:, sl].astype(NPBF)),
                "wk": np.ascontiguousarray(Wk[:, sl].astype(NPBF)),
                "wv": np.ascontiguousarray(Wv[:, sl].astype(NPBF)),
                "wo": np.ascontiguousarray(Wo[sl, :].astype(NPBF)),
                "bq": np.ascontiguousarray(bq[sl].reshape(MT, P).T, dtype=f32),
                "bk": np.ascontiguousarray(bk[sl].reshape(MT, P).T, dtype=f32),
                "bv": np.ascontiguousarray(bv[sl].reshape(1, DL), dtype=f32),
                "bo": np.ascontiguousarray(bo_core.reshape(1, D), dtype=f32),
            }
        )
    return in_maps, SK


def kernel(q, k, v, mask, Wq, bq, Wk, bk, Wv, bv, Wo, bo):
    from concourse.bass_utils import run_bass_kernel_spmd

    q = np.asarray(q)
    B, S, D = q.shape  # 4, 2048, 1024
    G = 2
    in_maps, SK = _shard_inputs(q, k, v, mask, Wq, bq, Wk, bk, Wv, bv, Wo, bo)
    nc = _get_nc(S, D, D // G, 64, SK)

    res = run_bass_kernel_spmd(nc, in_maps, core_ids=list(range(B * G)))
    parts = [r["out"] for r in res.results]
    outf = np.stack([parts[b * G] + parts[b * G + 1] for b in range(B)], axis=0)
    return outf.astype(np.float32)


# revision 12
# speedup vs baseline: 2.1825x; 1.1397x over previous
"""Multi-head attention (B=4, S=2048, D=1024, H=16) on 8 trn2 NeuronCores.

Sharding: 8 cores = 4 batches x 2 head-groups. Core c handles batch c//2 and
heads [8g, 8g+8) where g = c%2 (tensor-parallel: Wq/Wk/Wv column-sliced,
Wo row-sliced). Each core returns a partial output [S, D]; the host sums the
two head-group partials per batch.

Key-compaction: the mask drops a key entirely (exp(-1e9) == 0), so the host
gathers only the unmasked key rows of k/v per batch (padded to a 128-multiple
SK; pad slots are masked out on device). For the ~50% random mask this nearly
halves all K-side work (K/V projection, scores, exp, context).

All x / W tensors travel as bf16 (host casts): half the DMA traffic, PE
transposes run at 1 cycle/row instead of 2, and no on-device f32->bf16 casts.

Per-core dataflow (everything stays transposed until the output projection):
  xT tiles (PE transpose, bf16) -> Q.T/K.T = W.T @ X.T, V natural (ones
  column appended) -> scores.T = K @ Q.T -> exp+mask+scale in one ACT op
  -> ctxU.T = V'.T @ expS.T (last row = softmax denominator) -> normalize
  (fast-approx reciprocal) -> out = ctx.T.T @ Wo + bo.
"""

import sys

if "/opt/trn_rl_repo" not in sys.path:
    sys.path.append("/opt/trn_rl_repo")

import numpy as np
import ml_dtypes

import concourse.bass as bass
import concourse.bacc as bacc
import concourse.tile as tile
from concourse import mybir
from concourse.bass import ts

F32 = mybir.dt.float32
BF16 = mybir.dt.bfloat16
I32 = mybir.dt.int32
EXP = mybir.ActivationFunctionType.Exp

P = 128
NPBF = ml_dtypes.bfloat16


def build_nc(S=2048, D=1024, DL=512, HD=64, SK=1152):
    """Per-core Bass program. DL = local out dim (heads*HD), SK = key len."""
    ST = S // P  # q token tiles
    SKT = SK // P  # key token tiles
    KD = D // P  # contraction tiles over D
    MT = DL // P  # local d-col tiles
    HL = DL // HD  # local heads
    HPT = P // HD  # heads per 128-partition tile (2)
    NCH = min(512, S)  # projection token-chunk
    QS = min(1024, S)  # attention q superchunk (<=2 psum banks)
    QH = min(512, QS)  # one-bank half
    NH = QS // QH
    NQ = S // QS  # q-superchunks
    OC = min(512, D)  # out-proj col chunk
    scale = float(1.0 / (np.sqrt(np.float32(HD)) + 1e-8))

    # K-side token chunks (SK may not be a NCH multiple)
    kchunks = []
    o = 0
    while o < SK:
        w_ = min(NCH, SK - o)
        kchunks.append((o, w_))
        o += w_

    nc = bacc.Bacc("TRN2", target_bir_lowering=False, debug=False)

    xq = nc.dram_tensor("xq", [S, D], BF16, kind="ExternalInput")
    xk = nc.dram_tensor("xk", [SK, D], BF16, kind="ExternalInput")
    xv = nc.dram_tensor("xv", [SK, D], BF16, kind="ExternalInput")
    msk = nc.dram_tensor("msk", [P, SKT], I32, kind="ExternalInput")
    wq = nc.dram_tensor("wq", [D, DL], BF16, kind="ExternalInput")
    wk = nc.dram_tensor("wk", [D, DL], BF16, kind="ExternalInput")
    wv = nc.dram_tensor("wv", [D, DL], BF16, kind="ExternalInput")
    wo = nc.dram_tensor("wo", [DL, D], BF16, kind="ExternalInput")
    bq = nc.dram_tensor("bq", [P, MT], F32, kind="ExternalInput")
    bk = nc.dram_tensor("bk", [P, MT], F32, kind="ExternalInput")
    bv = nc.dram_tensor("bv", [1, DL], F32, kind="ExternalInput")
    bo = nc.dram_tensor("bo", [1, D], F32, kind="ExternalInput")
    out = nc.dram_tensor("out", [S, D], F32, kind="ExternalOutput")

    with tile.TileContext(nc) as tc, nc.allow_low_precision("bf16 operands are rounded by design"):
        with (
            tc.tile_pool(name="pers", bufs=1) as pers,
            tc.tile_pool(name="wpool", bufs=2) as wpool,
            tc.tile_pool(name="xt", bufs=3) as xt_pool,
            tc.tile_pool(name="exp", bufs=10) as ex_pool,
            tc.tile_pool(name="osb", bufs=3) as osb_pool,
            tc.tile_pool(name="small", bufs=2) as small,
        ):
            # ---- constants (only what the first K-proj chunk needs; the
            # rest is emitted after it so the first DMAs/gpsimd ops on the
            # queues are the ones the critical path waits on) ----
            bks = pers.tile([P, MT], F32, tag="bks")
            nc.sync.dma_start(bks[:], bk[:, :])

            mi = pers.tile([P, SKT], I32, tag="mi")
            mf = pers.tile([P, SKT], F32, tag="mf")
            mb = pers.tile([P, SKT], F32, tag="mb")
            bqs = pers.tile([P, MT], F32, tag="bqs")
            bvs = pers.tile([1, DL], F32, tag="bvs")
            bos = pers.tile([1, D], F32, tag="bos")
            bvb = pers.tile([P, DL], F32, tag="bvb")
            bob = pers.tile([P, D], F32, tag="bob")

            def late_consts():
                nc.sync.dma_start(mi[:], msk[:, :])
                nc.vector.tensor_copy(out=mf[:], in_=mi[:])
                nc.vector.tensor_scalar_mul(mb[:], mf[:], -1.0e9)
                nc.sync.dma_start(bqs[:], bq[:, :])
                nc.sync.dma_start(bvs[:], bv[:, :])
                nc.sync.dma_start(bos[:], bo[:, :])
                nc.gpsimd.partition_broadcast(bvb[:], bvs[0:1, :])
                nc.gpsimd.partition_broadcast(bob[:], bos[0:1, :])
                for t in range(SKT):
                    nc.gpsimd.memset(VP[t][:], 1.0)

            # persistent activation stores
            KT = [pers.tile([P, SK], BF16, tag=f"kt{m}", name=f"kt{m}") for m in range(MT)]
            QT = [pers.tile([P, S], BF16, tag=f"qt{m}", name=f"qt{m}") for m in range(MT)]
            CT = [pers.tile([P, S], BF16, tag=f"ct{m}", name=f"ct{m}") for m in range(MT)]
            VP = [pers.tile([P, HL * (HD + 1)], BF16, tag=f"vp{t}", name=f"vp{t}") for t in range(SKT)]

            def load_w(wdram):
                w = wpool.tile([P, KD, DL], BF16, tag="w", name="w")
                nc.sync.dma_start(w[:], wdram.rearrange("(k p) n -> p k n", p=P))
                return w

            wos = pers.tile([P, MT, D], BF16, tag="wos")

            def proj_units(xdram, wsb, bias_sb, dst_tiles, tok0, ntok, acc_pool):
                """dst[m][:, tok0:tok0+ntok] = (x @ w + b).T; yields at unit edges.

                x.T arrives via the DMA-engine transpose (xt[p, kk, t] =
                x[t, kk*128+p]) — no PE transposes, no PSUM staging."""
                xt = xt_pool.tile([P, KD, NCH], BF16, tag="xt", name="xt")
                nc.sync.dma_start_transpose(
                    xt[:, :, 0:ntok], xdram[tok0 : tok0 + ntok, :]
                )
                yield
                for m in range(MT):
                    acc = acc_pool.tile([P, NCH], F32, tag="acc")
                    for kk in range(KD):
                        nc.tensor.matmul(
                            acc[:, 0:ntok],
                            lhsT=wsb[:, kk, ts(m, P)],
                            rhs=xt[:, kk, 0:ntok],
                            start=(kk == 0),
                            stop=(kk == KD - 1),
                        )
                    nc.vector.tensor_scalar_add(
                        dst_tiles[m][:, tok0 : tok0 + ntok], acc[:, 0:ntok], bias_sb[:, m : m + 1]
                    )
                    yield

            def proj_T(xdram, wsb, bias_sb, dst_tiles, tok0, ntok, acc_pool):
                for _ in proj_units(
                    xdram, wsb, bias_sb, dst_tiles, tok0, ntok, acc_pool
                ):
                    pass

            def vproj(wsb, acc_pool):
                """VP[t][:, h*(HD+1):+HD] = (xv @ wv + bv)[t-tile, h-slice]."""
                for t in range(SKT):
                    xt = xt_pool.tile([P, KD, NCH], BF16, tag="xt", name="xt")
                    nc.sync.dma_start_transpose(xt[:, :, 0:P], xv[ts(t, P), :])
                    acc = acc_pool.tile([P, DL], F32, tag="acc")
                    for kk in range(KD):
                        nc.tensor.matmul(
                            acc[:],
                            lhsT=xt[:, kk, 0:P],
                            rhs=wsb[:, kk, :],
                            start=(kk == 0),
                            stop=(kk == KD - 1),
                        )
                    for h in range(HL):
                        nc.vector.tensor_add(
                            VP[t][:, h * (HD + 1) : h * (HD + 1) + HD],
                            acc[:, ts(h, HD)],
                            bvb[:, ts(h, HD)],
                        )

            def attention(qq, sc_pool, cx_pool, filler=None, pump_every=8):
                it = 0
                pending = []  # deferred normalize work (recip/broadcast/mul)
                for hp in range(HL // HPT):  # head pairs share a KT/QT tile
                    for q5 in range(NH):
                        col0 = qq * QS + q5 * QH
                        cxs = [
                            cx_pool.tile([HD + 1, QH], F32, tag="cx", name="cx")
                            for _ in range(HPT)
                        ]
                        for kt in range(SKT):
                            # one PSUM supertile holds both heads' score chunk;
                            # the two K=64 matmuls run concurrently (row groups
                            # 0-63 / 64-127), one ACT exp covers both
                            sc = sc_pool.tile([P, HPT * QH], F32, tag="sc")
                            for u in range(HPT):
                                mo = u * HD
                                nc.tensor.matmul(
                                    sc[:, ts(u, QH)],
                                    lhsT=KT[hp][mo : mo + HD, ts(kt, P)],
                                    rhs=QT[hp][mo : mo + HD, col0 : col0 + QH],
                                    start=True,
                                    stop=True,
                                )
                            ex = ex_pool.tile([P, HPT * QH], BF16, tag="ex")
                            nc.scalar.activation(
                                ex[:], sc[:], EXP, bias=mb[:, kt : kt + 1], scale=scale
                            )
                            for u in range(HPT):
                                h = hp * HPT + u
                                nc.tensor.matmul(
                                    cxs[u][:],
                                    lhsT=VP[kt][:, h * (HD + 1) : (h + 1) * (HD + 1)],
                                    rhs=ex[:, ts(u, QH)],
                                    start=(kt == 0),
                                    stop=(kt == SKT - 1),
                                )
                            it += 1
                            if filler is not None and it % pump_every == 0:
                                next(filler, None)
                        prev_tails = pending
                        pending = []
                        for u in range(HPT):
                            mo = u * HD
                            # the cheap DVE copy (emitted now, ahead of the
                            # previous unit's reciprocals in DVE order) frees
                            # the ctx PSUM slot; recip/broadcast/mul are
                            # deferred one unit so nothing waits on them
                            stg = small.tile([HD + 1, QH], F32, tag="stg", name="stg", bufs=4)
                            nc.vector.tensor_copy(out=stg[:], in_=cxs[u][:])

                            def tail(hp=hp, mo=mo, col0=col0, stg=stg):
                                # denominator row lives at partition HD; the
                                # custom gpsimd/DVE ops read absolute partition
                                # 0 on HW, so DMA it down to a base-0 tile first
                                rec = small.tile([1, QH], F32, tag="rec", name="rec", bufs=2)
                                nc.sync.dma_start(rec[0:1, :], stg[HD : HD + 1, :])
                                dnb = small.tile([HD, QH], F32, tag="dnb", name="dnb", bufs=2)
                                nc.gpsimd.partition_broadcast(dnb[:], rec[0:1, :])
                                bcs = small.tile([HD, QH], F32, tag="bcs", bufs=2)
                                nc.vector.reciprocal_approx_fast(bcs[:], dnb[:])
                                if mo == 0:
                                    nc.vector.tensor_mul(
                                        CT[hp][0:HD, col0 : col0 + QH],
                                        stg[0:HD, :],
                                        bcs[:],
                                    )
                                else:
                                    tmp = small.tile([HD, QH], BF16, tag="tmp")
                                    nc.vector.tensor_mul(tmp[:], stg[0:HD, :], bcs[:])
                                    nc.sync.dma_start(
                                        CT[hp][mo : mo + HD, col0 : col0 + QH], tmp[:]
                                    )

                            pending.append(tail)
                        for fn in prev_tails:
                            fn()

                for fn in pending:
                    fn()

            def outproj_units(qq, acc_pool):
                t0 = qq * (QS // P)
                for t in range(t0, t0 + QS // P):
                    for c in range(D // OC):
                        po = acc_pool.tile([P, OC], F32, tag="acc", name="po")
                        for dd in range(MT):
                            nc.tensor.matmul(
                                po[:],
                                lhsT=CT[dd][:, ts(t, P)],
                                rhs=wos[:, dd, ts(c, OC)],
                                start=(dd == 0),
                                stop=(dd == MT - 1),
                            )
                        osb = osb_pool.tile([P, OC], F32, tag="osb")
                        nc.vector.tensor_add(osb[:], po[:], bob[:, ts(c, OC)])
                        nc.sync.dma_start(out[ts(t, P), ts(c, OC)], osb[:])
                        yield

            # ---- phase 1: K.T and V' (full-S prerequisites of attention) ----
            with tc.tile_pool(name="ps1acc", bufs=6, space="PSUM") as ps1acc:
                wks = load_w(wk)
                for ci, (tok0, ntok) in enumerate(kchunks):
                    proj_T(xk, wks, bks, KT, tok0, ntok, ps1acc)
                    if ci == 0:
                        late_consts()
                wvs = load_w(wv)
                vproj(wvs, ps1acc)

            # ---- phase 2: Q.T chunks, attention, out-proj ----
            with (
                tc.tile_pool(name="ps2acc", bufs=2, space="PSUM") as ps2acc,
                tc.tile_pool(name="ps2sc", bufs=2, space="PSUM") as ps2sc,
                tc.tile_pool(name="ps2cx", bufs=2, space="PSUM") as ps2cx,
            ):
                wqs = load_w(wq)
                nc.sync.dma_start(wos[:], wo.rearrange("(m p) n -> p m n", p=P))
                CPQ = QS // NCH  # projection chunks per q-superchunk
                from itertools import chain

                n_att_its = (HL // HPT) * NH * SKT
                for nch in range(CPQ):
                    proj_T(xq, wqs, bqs, QT, nch * NCH, NCH, ps2acc)
                for qq in range(NQ):
                    if qq + 1 < NQ:
                        filler = chain.from_iterable(
                            proj_units(xq, wqs, bqs, QT, nch * NCH, NCH, ps2acc)
                            for nch in range((qq + 1) * CPQ, (qq + 2) * CPQ)
                        )
                        n_units = CPQ * (1 + MT)
                    elif qq >= 1:
                        filler = outproj_units(qq - 1, ps2acc)
                        n_units = (QS // P) * (D // OC)
                    else:
                        filler = None
                        n_units = 1
                    attention(
                        qq,
                        ps2sc,
                        ps2cx,
                        filler,
                        pump_every=max(1, n_att_its // max(n_units, 1)),
                    )
                    if filler is not None:
                        for _ in filler:
                            pass
                if NQ >= 2:
                    for qq in range(NQ - 2):
                        for _ in outproj_units(qq, ps2acc):
                            pass

            # ---- phase 3: final out-proj; psum depth 2 is enough for the
            # epilogue (vector add + store) to trail by less than a unit ----
            with tc.tile_pool(name="ps3", bufs=2, space="PSUM") as ps3:
                for _ in outproj_units(NQ - 1, ps3):
                    pass

    nc.compile()
    return nc


_NC_CACHE = {}


def _get_nc(S, D, DL, HD, SK):
    key = (S, D, DL, HD, SK)
    if key not in _NC_CACHE:
        _NC_CACHE[key] = build_nc(S, D, DL, HD, SK)
    return _NC_CACHE[key]


def _shard_inputs(q, k, v, mask, Wq, bq, Wk, bk, Wv, bv, Wo, bo):
    q, k, v = np.asarray(q), np.asarray(k), np.asarray(v)
    mask = np.asarray(mask)
    Wq, Wk, Wv, Wo = np.asarray(Wq), np.asarray(Wk), np.asarray(Wv), np.asarray(Wo)
    bq, bk, bv, bo = np.asarray(bq), np.asarray(bk), np.asarray(bv), np.asarray(bo)

    B, S, D = q.shape  # 4, 2048, 1024
    G = 2  # head-groups (tensor-parallel factor); B*G = 8 cores
    DL = D // G
    MT = DL // P

    # key compaction: gather unmasked key rows, pad to a 128-multiple
    keep = [np.flatnonzero(mask[b, 0, 0] == 0) for b in range(B)]
    maxc = max(max((len(ix) for ix in keep), default=1), 1)
    SK = min(S, ((maxc + P - 1) // P) * P)
    SKT = SK // P

    f32 = np.float32
    xk_c, xv_c, msk_c = [], [], []
    for b in range(B):
        if SK == S:
            # fallback: no compaction, original order + original mask
            xk_c.append(np.ascontiguousarray(k[b], dtype=NPBF))
            xv_c.append(np.ascontiguousarray(v[b], dtype=NPBF))
            msk_c.append(
                np.ascontiguousarray(mask[b, 0, 0].reshape(SKT, P).T, dtype=np.int32)
            )
        else:
            ix = keep[b][:SK]
            n = len(ix)
            kb = np.zeros((SK, D), dtype=NPBF)
            vb = np.zeros((SK, D), dtype=NPBF)
            kb[:n] = k[b][ix].astype(NPBF)
            vb[:n] = v[b][ix].astype(NPBF)
            mb_ = np.zeros((SK,), dtype=np.int32)
            mb_[n:] = 1
            xk_c.append(kb)
            xv_c.append(vb)
            msk_c.append(np.ascontiguousarray(mb_.reshape(SKT, P).T, dtype=np.int32))

    xq_b = [np.ascontiguousarray(q[b], dtype=NPBF) for b in range(B)]

    in_maps = []
    for c in range(B * G):
        b, g = c // G, c % G
        sl = slice(g * DL, (g + 1) * DL)
        bo_core = bo if g == 0 else np.zeros_like(bo)
        in_maps.append(
            {
                "xq": xq_b[b],
                "xk": xk_c[b],
                "xv": xv_c[b],
                "msk": msk_c[b],
                "wq": np.ascontiguousarray(Wq[:, sl].astype(NPBF)),
                "wk": np.ascontiguousarray(Wk[:, sl].astype(NPBF)),
                "wv": np.ascontiguousarray(Wv[:, sl].astype(NPBF)),
                "wo": np.ascontiguousarray(Wo[sl, :].astype(NPBF)),
                "bq": np.ascontiguousarray(bq[sl].reshape(MT, P).T, dtype=f32),
                "bk": np.ascontiguousarray(bk[sl].reshape(MT, P).T, dtype=f32),
                "bv": np.ascontiguousarray(bv[sl].reshape(1, DL), dtype=f32),
                "bo": np.ascontiguousarray(bo_core.reshape(1, D), dtype=f32),
            }
        )
    return in_maps, SK


def kernel(q, k, v, mask, Wq, bq, Wk, bk, Wv, bv, Wo, bo):
    from concourse.bass_utils import run_bass_kernel_spmd

    q = np.asarray(q)
    B, S, D = q.shape  # 4, 2048, 1024
    G = 2
    in_maps, SK = _shard_inputs(q, k, v, mask, Wq, bq, Wk, bk, Wv, bv, Wo, bo)
    nc = _get_nc(S, D, D // G, 64, SK)

    res = run_bass_kernel_spmd(nc, in_maps, core_ids=list(range(B * G)))
    parts = [r["out"] for r in res.results]
    outf = np.stack([parts[b * G] + parts[b * G + 1] for b in range(B)], axis=0)
    return outf.astype(np.float32)


# revision 16
# speedup vs baseline: 2.2159x; 1.0153x over previous
"""Multi-head attention (B=4, S=2048, D=1024, H=16) on 8 trn2 NeuronCores.

Sharding: 8 cores = 4 batches x 2 head-groups. Core c handles batch c//2 and
heads [8g, 8g+8) where g = c%2 (tensor-parallel: Wq/Wk/Wv column-sliced,
Wo row-sliced). Each core returns a partial output [S, D]; the host sums the
two head-group partials per batch.

Key-compaction: the mask drops a key entirely (exp(-1e9) == 0), so the host
gathers only the unmasked key rows of k/v per batch (padded to a 128-multiple
SK; pad slots are masked out on device). For the ~50% random mask this nearly
halves all K-side work (K/V projection, scores, exp, context).

All x / W tensors travel as bf16 (host casts): half the DMA traffic, PE
transposes run at 1 cycle/row instead of 2, and no on-device f32->bf16 casts.

Per-core dataflow (everything stays transposed until the output projection):
  xT tiles (PE transpose, bf16) -> Q.T/K.T = W.T @ X.T, V natural (ones
  column appended) -> scores.T = K @ Q.T -> exp+mask+scale in one ACT op
  -> ctxU.T = V'.T @ expS.T (last row = softmax denominator) -> normalize
  (fast-approx reciprocal) -> out = ctx.T.T @ Wo + bo.
"""

import sys

if "/opt/trn_rl_repo" not in sys.path:
    sys.path.append("/opt/trn_rl_repo")

import numpy as np
import ml_dtypes

import concourse.bass as bass
import concourse.bacc as bacc
import concourse.tile as tile
from concourse import mybir
from concourse.bass import ts

F32 = mybir.dt.float32
BF16 = mybir.dt.bfloat16
I32 = mybir.dt.int32
EXP = mybir.ActivationFunctionType.Exp

P = 128
NPBF = ml_dtypes.bfloat16


def build_nc(S=2048, D=1024, DL=512, HD=64, SK=1152):
    """Per-core Bass program. DL = local out dim (heads*HD), SK = key len."""
    ST = S // P  # q token tiles
    SKT = SK // P  # key token tiles
    KD = D // P  # contraction tiles over D
    MT = DL // P  # local d-col tiles
    HL = DL // HD  # local heads
    HPT = P // HD  # heads per 128-partition tile (2)
    NCH = min(512, S)  # projection token-chunk
    QS = min(1024, S)  # attention q superchunk (<=2 psum banks)
    QH = min(512, QS)  # one-bank half
    NH = QS // QH
    NQ = S // QS  # q-superchunks
    OC = min(512, D)  # out-proj col chunk
    scale = float(1.0 / (np.sqrt(np.float32(HD)) + 1e-8))

    # K-side token chunks (SK may not be a NCH multiple)
    kchunks = []
    o = 0
    while o < SK:
        w_ = min(NCH, SK - o)
        kchunks.append((o, w_))
        o += w_

    nc = bacc.Bacc("TRN2", target_bir_lowering=False, debug=False)

    xq = nc.dram_tensor("xq", [S, D], BF16, kind="ExternalInput")
    xk = nc.dram_tensor("xk", [SK, D], BF16, kind="ExternalInput")
    xv = nc.dram_tensor("xv", [SK, D], BF16, kind="ExternalInput")
    msk = nc.dram_tensor("msk", [P, SKT], I32, kind="ExternalInput")
    wq = nc.dram_tensor("wq", [D, DL], BF16, kind="ExternalInput")
    wk = nc.dram_tensor("wk", [D, DL], BF16, kind="ExternalInput")
    wv = nc.dram_tensor("wv", [D, DL], BF16, kind="ExternalInput")
    wo = nc.dram_tensor("wo", [DL, D], BF16, kind="ExternalInput")
    bq = nc.dram_tensor("bq", [P, MT], F32, kind="ExternalInput")
    bk = nc.dram_tensor("bk", [P, MT], F32, kind="ExternalInput")
    bv = nc.dram_tensor("bv", [1, DL], F32, kind="ExternalInput")
    bo = nc.dram_tensor("bo", [1, D], F32, kind="ExternalInput")
    out = nc.dram_tensor("out", [S, D], F32, kind="ExternalOutput")

    with tile.TileContext(nc) as tc, nc.allow_low_precision("bf16 operands are rounded by design"):
        with (
            tc.tile_pool(name="pers", bufs=1) as pers,
            tc.tile_pool(name="wpool", bufs=2) as wpool,
            tc.tile_pool(name="xt", bufs=4) as xt_pool,
            tc.tile_pool(name="exp", bufs=10) as ex_pool,
            tc.tile_pool(name="osb", bufs=3) as osb_pool,
            tc.tile_pool(name="small", bufs=2) as small,
        ):
            # ---- constants. All weight/bias/mask loads go through the
            # Scalar engine's DMA queue so the Sync queue (x transposes,
            # stores) is never blocked behind them. Everything except bks
            # is emitted after the first K-proj chunk so the first Sync
            # DMA is the one the critical path waits on. ----
            bks = pers.tile([P, MT], F32, tag="bks")
            nc.scalar.dma_start(bks[:], bk[:, :])

            mi = pers.tile([P, SKT], I32, tag="mi")
            mf = pers.tile([P, SKT], F32, tag="mf")
            mb = pers.tile([P, SKT], F32, tag="mb")
            bqs = pers.tile([P, MT], F32, tag="bqs")
            bvs = pers.tile([1, DL], F32, tag="bvs")
            bos = pers.tile([1, D], F32, tag="bos")
            bvb = pers.tile([P, DL], F32, tag="bvb")
            bob = pers.tile([P, D], F32, tag="bob")

            def late_consts():
                nc.scalar.dma_start(mi[:], msk[:, :])
                nc.vector.tensor_copy(out=mf[:], in_=mi[:])
                nc.vector.tensor_scalar_mul(mb[:], mf[:], -1.0e9)
                nc.scalar.dma_start(bqs[:], bq[:, :])
                nc.scalar.dma_start(bvs[:], bv[:, :])
                nc.scalar.dma_start(bos[:], bo[:, :])
                nc.gpsimd.partition_broadcast(bvb[:], bvs[0:1, :])
                nc.gpsimd.partition_broadcast(bob[:], bos[0:1, :])
                for t in range(SKT):
                    nc.gpsimd.memset(VP[t][:], 1.0)

            # persistent activation stores
            KT = [pers.tile([P, SK], BF16, tag=f"kt{m}", name=f"kt{m}") for m in range(MT)]
            QT = [pers.tile([P, S], BF16, tag=f"qt{m}", name=f"qt{m}") for m in range(MT)]
            CT = [pers.tile([P, S], BF16, tag=f"ct{m}", name=f"ct{m}") for m in range(MT)]
            VP = [pers.tile([P, HL * (HD + 1)], BF16, tag=f"vp{t}", name=f"vp{t}") for t in range(SKT)]

            def load_w(wdram):
                w = wpool.tile([P, KD, DL], BF16, tag="w", name="w")
                nc.scalar.dma_start(w[:], wdram.rearrange("(k p) n -> p k n", p=P))
                return w

            wos = pers.tile([P, MT, D], BF16, tag="wos")

            def xT_load(xdram, tok0, ntok):
                """xt[p, kk, t] = x[tok0+t, kk*128+p] via the DMA-engine
                transpose — no PE transposes, no PSUM staging."""
                xt = xt_pool.tile([P, KD, NCH], BF16, tag="xt", name="xt")
                nc.sync.dma_start_transpose(
                    xt[:, :, 0:ntok], xdram[tok0 : tok0 + ntok, :]
                )
                return xt

            def proj_mm_units(xt, wsb, bias_sb, dst_tiles, tok0, ntok, acc_pool):
                """dst[m][:, tok0:tok0+ntok] = (x @ w + b).T; yields per m."""
                for m in range(MT):
                    acc = acc_pool.tile([P, NCH], F32, tag="acc")
                    for kk in range(KD):
                        nc.tensor.matmul(
                            acc[:, 0:ntok],
                            lhsT=wsb[:, kk, ts(m, P)],
                            rhs=xt[:, kk, 0:ntok],
                            start=(kk == 0),
                            stop=(kk == KD - 1),
                        )
                    nc.vector.tensor_scalar_add(
                        dst_tiles[m][:, tok0 : tok0 + ntok], acc[:, 0:ntok], bias_sb[:, m : m + 1]
                    )
                    yield

            def proj_units(xdram, wsb, bias_sb, dst_tiles, tok0, ntok, acc_pool):
                xt = xT_load(xdram, tok0, ntok)
                yield
                yield from proj_mm_units(
                    xt, wsb, bias_sb, dst_tiles, tok0, ntok, acc_pool
                )

            def proj_T(xdram, wsb, bias_sb, dst_tiles, tok0, ntok, acc_pool):
                for _ in proj_units(
                    xdram, wsb, bias_sb, dst_tiles, tok0, ntok, acc_pool
                ):
                    pass

            def vproj(wsb, acc_pool):
                """VP[t][:, h*(HD+1):+HD] = (xv @ wv + bv)[t-tile, h-slice]."""
                for t in range(SKT):
                    xt = xT_load(xv, t * P, P)
                    acc = acc_pool.tile([P, DL], F32, tag="acc")
                    for kk in range(KD):
                        nc.tensor.matmul(
                            acc[:],
                            lhsT=xt[:, kk, 0:P],
                            rhs=wsb[:, kk, :],
                            start=(kk == 0),
                            stop=(kk == KD - 1),
                        )
                    for h in range(HL):
                        nc.vector.tensor_add(
                            VP[t][:, h * (HD + 1) : h * (HD + 1) + HD],
                            acc[:, ts(h, HD)],
                            bvb[:, ts(h, HD)],
                        )

            def attention(qq, sc_pool, cx_pool, filler=None, pump_every=8):
                it = 0
                pending = []  # deferred normalize work (recip/broadcast/mul)
                for hp in range(HL // HPT):  # head pairs share a KT/QT tile
                    for q5 in range(NH):
                        col0 = qq * QS + q5 * QH
                        cxs = [
                            cx_pool.tile([HD + 1, QH], F32, tag="cx", name="cx")
                            for _ in range(HPT)
                        ]
                        for kt in range(SKT):
                            # one PSUM supertile holds both heads' score chunk;
                            # the two K=64 matmuls run concurrently (row groups
                            # 0-63 / 64-127), one ACT exp covers both
                            sc = sc_pool.tile([P, HPT * QH], F32, tag="sc")
                            for u in range(HPT):
                                mo = u * HD
                                nc.tensor.matmul(
                                    sc[:, ts(u, QH)],
                                    lhsT=KT[hp][mo : mo + HD, ts(kt, P)],
                                    rhs=QT[hp][mo : mo + HD, col0 : col0 + QH],
                                    start=True,
                                    stop=True,
                                )
                            ex = ex_pool.tile([P, HPT * QH], BF16, tag="ex")
                            nc.scalar.activation(
                                ex[:], sc[:], EXP, bias=mb[:, kt : kt + 1], scale=scale
                            )
                            for u in range(HPT):
                                h = hp * HPT + u
                                nc.tensor.matmul(
                                    cxs[u][:],
                                    lhsT=VP[kt][:, h * (HD + 1) : (h + 1) * (HD + 1)],
                                    rhs=ex[:, ts(u, QH)],
                                    start=(kt == 0),
                                    stop=(kt == SKT - 1),
                                )
                            it += 1
                            if filler is not None and it % pump_every == 0:
                                next(filler, None)
                        prev_tails = pending
                        pending = []
                        for u in range(HPT):
                            mo = u * HD
                            # the cheap DVE copy (emitted now, ahead of the
                            # previous unit's reciprocals in DVE order) frees
                            # the ctx PSUM slot; recip/broadcast/mul are
                            # deferred one unit so nothing waits on them
                            stg = small.tile([HD + 1, QH], F32, tag="stg", name="stg", bufs=4)
                            nc.vector.tensor_copy(out=stg[:], in_=cxs[u][:])

                            def tail(hp=hp, mo=mo, col0=col0, stg=stg):
                                # denominator row lives at partition HD; the
                                # custom gpsimd/DVE ops read absolute partition
                                # 0 on HW, so DMA it down to a base-0 tile first
                                rec = small.tile([1, QH], F32, tag="rec", name="rec", bufs=2)
                                nc.sync.dma_start(rec[0:1, :], stg[HD : HD + 1, :])
                                dnb = small.tile([HD, QH], F32, tag="dnb", name="dnb", bufs=2)
                                nc.gpsimd.partition_broadcast(dnb[:], rec[0:1, :])
                                bcs = small.tile([HD, QH], F32, tag="bcs", bufs=2)
                                nc.vector.reciprocal_approx_fast(bcs[:], dnb[:])
                                if mo == 0:
                                    nc.vector.tensor_mul(
                                        CT[hp][0:HD, col0 : col0 + QH],
                                        stg[0:HD, :],
                                        bcs[:],
                                    )
                                else:
                                    tmp = small.tile([HD, QH], BF16, tag="tmp")
                                    nc.vector.tensor_mul(tmp[:], stg[0:HD, :], bcs[:])
                                    nc.sync.dma_start(
                                        CT[hp][mo : mo + HD, col0 : col0 + QH], tmp[:]
                                    )

                            pending.append(tail)
                        for fn in prev_tails:
                            fn()

                for fn in pending:
                    fn()

            def outproj_units(qq, acc_pool):
                t0 = qq * (QS // P)
                for t in range(t0, t0 + QS // P):
                    for c in range(D // OC):
                        po = acc_pool.tile([P, OC], F32, tag="acc", name="po")
                        for dd in range(MT):
                            nc.tensor.matmul(
                                po[:],
                                lhsT=CT[dd][:, ts(t, P)],
                                rhs=wos[:, dd, ts(c, OC)],
                                start=(dd == 0),
                                stop=(dd == MT - 1),
                            )
                        osb = osb_pool.tile([P, OC], F32, tag="osb")
                        nc.vector.tensor_add(osb[:], po[:], bob[:, ts(c, OC)])
                        nc.sync.dma_start(out[ts(t, P), ts(c, OC)], osb[:])
                        yield

            CPQ = QS // NCH  # projection chunks per q-superchunk

            # ---- phase 1: K.T and V' (full-S prerequisites of attention) ----
            with tc.tile_pool(name="ps1acc", bufs=6, space="PSUM") as ps1acc:
                xt0 = xT_load(xk, *kchunks[0])
                wks = load_w(wk)
                for ci, (tok0, ntok) in enumerate(kchunks):
                    xt = xt0 if ci == 0 else xT_load(xk, tok0, ntok)
                    for _ in proj_mm_units(xt, wks, bks, KT, tok0, ntok, ps1acc):
                        pass
                    if ci == 0:
                        late_consts()
                wvs = load_w(wv)
                vproj(wvs, ps1acc)
                # prefetch the first q-superchunk's transposes + weights so
                # phase 2 starts on a hot path
                wqs = load_w(wq)
                nc.scalar.dma_start(wos[:], wo.rearrange("(m p) n -> p m n", p=P))
                qxt0 = [xT_load(xq, nch * NCH, NCH) for nch in range(CPQ)]

            # ---- phase 2: Q.T chunks, attention, out-proj ----
            with (
                tc.tile_pool(name="ps2acc", bufs=2, space="PSUM") as ps2acc,
                tc.tile_pool(name="ps2sc", bufs=2, space="PSUM") as ps2sc,
                tc.tile_pool(name="ps2cx", bufs=2, space="PSUM") as ps2cx,
            ):
                from itertools import chain

                n_att_its = (HL // HPT) * NH * SKT
                for nch in range(CPQ):
                    for _ in proj_mm_units(
                        qxt0[nch], wqs, bqs, QT, nch * NCH, NCH, ps2acc
                    ):
                        pass
                for qq in range(NQ):
                    if qq + 1 < NQ:
                        filler = chain.from_iterable(
                            proj_units(xq, wqs, bqs, QT, nch * NCH, NCH, ps2acc)
                            for nch in range((qq + 1) * CPQ, (qq + 2) * CPQ)
                        )
                        n_units = CPQ * (1 + MT)
                    elif qq >= 1:
                        filler = outproj_units(qq - 1, ps2acc)
                        n_units = (QS // P) * (D // OC)
                    else:
                        filler = None
                        n_units = 1
                    attention(
                        qq,
                        ps2sc,
                        ps2cx,
                        filler,
                        pump_every=max(1, n_att_its // max(n_units, 1)),
                    )
                    if filler is not None:
                        for _ in filler:
                            pass
                if NQ >= 2:
                    for qq in range(NQ - 2):
                        for _ in outproj_units(qq, ps2acc):
                            pass

            # ---- phase 3: final out-proj; psum depth 2 is enough for the
            # epilogue (vector add + store) to trail by less than a unit ----
            with tc.tile_pool(name="ps3", bufs=2, space="PSUM") as ps3:
                for _ in outproj_units(NQ - 1, ps3):
                    pass

    nc.compile()
    return nc


_NC_CACHE = {}


def _get_nc(S, D, DL, HD, SK):
    key = (S, D, DL, HD, SK)
    if key not in _NC_CACHE:
        _NC_CACHE[key] = build_nc(S, D, DL, HD, SK)
    return _NC_CACHE[key]


def _shard_inputs(q, k, v, mask, Wq, bq, Wk, bk, Wv, bv, Wo, bo):
    q, k, v = np.asarray(q), np.asarray(k), np.asarray(v)
    mask = np.asarray(mask)
    Wq, Wk, Wv, Wo = np.asarray(Wq), np.asarray(Wk), np.asarray(Wv), np.asarray(Wo)
    bq, bk, bv, bo = np.asarray(bq), np.asarray(bk), np.asarray(bv), np.asarray(bo)

    B, S, D = q.shape  # 4, 2048, 1024
    G = 2  # head-groups (tensor-parallel factor); B*G = 8 cores
    DL = D // G
    MT = DL // P

    # key compaction: gather unmasked key rows, pad to a 128-multiple
    keep = [np.flatnonzero(mask[b, 0, 0] == 0) for b in range(B)]
    maxc = max(max((len(ix) for ix in keep), default=1), 1)
    SK = min(S, ((maxc + P - 1) // P) * P)
    SKT = SK // P

    f32 = np.float32
    xk_c, xv_c, msk_c = [], [], []
    for b in range(B):
        if SK == S:
            # fallback: no compaction, original order + original mask
            xk_c.append(np.ascontiguousarray(k[b], dtype=NPBF))
            xv_c.append(np.ascontiguousarray(v[b], dtype=NPBF))
            msk_c.append(
                np.ascontiguousarray(mask[b, 0, 0].reshape(SKT, P).T, dtype=np.int32)
            )
        else:
            ix = keep[b][:SK]
            n = len(ix)
            kb = np.zeros((SK, D), dtype=NPBF)
            vb = np.zeros((SK, D), dtype=NPBF)
            kb[:n] = k[b][ix].astype(NPBF)
            vb[:n] = v[b][ix].astype(NPBF)
            mb_ = np.zeros((SK,), dtype=np.int32)
            mb_[n:] = 1
            xk_c.append(kb)
            xv_c.append(vb)
            msk_c.append(np.ascontiguousarray(mb_.reshape(SKT, P).T, dtype=np.int32))

    xq_b = [np.ascontiguousarray(q[b], dtype=NPBF) for b in range(B)]

    in_maps = []
    for c in range(B * G):
        b, g = c // G, c % G
        sl = slice(g * DL, (g + 1) * DL)
        bo_core = bo if g == 0 else np.zeros_like(bo)
        in_maps.append(
            {
                "xq": xq_b[b],
                "xk": xk_c[b],
                "xv": xv_c[b],
                "msk": msk_c[b],
                "wq": np.ascontiguousarray(Wq[:, sl].astype(NPBF)),
                "wk": np.ascontiguousarray(Wk[:, sl].astype(NPBF)),
                "wv": np.ascontiguousarray(Wv[:, sl].astype(NPBF)),
                "wo": np.ascontiguousarray(Wo[sl, :].astype(NPBF)),
                "bq": np.ascontiguousarray(bq[sl].reshape(MT, P).T, dtype=f32),
                "bk": np.ascontiguousarray(bk[sl].reshape(MT, P).T, dtype=f32),
                "bv": np.ascontiguousarray(bv[sl].reshape(1, DL), dtype=f32),
                "bo": np.ascontiguousarray(bo_core.reshape(1, D), dtype=f32),
            }
        )
    return in_maps, SK


def kernel(q, k, v, mask, Wq, bq, Wk, bk, Wv, bv, Wo, bo):
    from concourse.bass_utils import run_bass_kernel_spmd

    q = np.asarray(q)
    B, S, D = q.shape  # 4, 2048, 1024
    G = 2
    in_maps, SK = _shard_inputs(q, k, v, mask, Wq, bq, Wk, bk, Wv, bv, Wo, bo)
    nc = _get_nc(S, D, D // G, 64, SK)

    res = run_bass_kernel_spmd(nc, in_maps, core_ids=list(range(B * G)))
    parts = [r["out"] for r in res.results]
    outf = np.stack([parts[b * G] + parts[b * G + 1] for b in range(B)], axis=0)
    return outf.astype(np.float32)


# revision 19
# speedup vs baseline: 2.2307x; 1.0067x over previous
"""Multi-head attention (B=4, S=2048, D=1024, H=16) on 8 trn2 NeuronCores.

Sharding: 8 cores = 4 batches x 2 head-groups. Core c handles batch c//2 and
heads [8g, 8g+8) where g = c%2 (tensor-parallel: Wq/Wk/Wv column-sliced,
Wo row-sliced). Each core returns a partial output [S, D]; the host sums the
two head-group partials per batch.

Key-compaction: the mask drops a key entirely (exp(-1e9) == 0), so the host
gathers only the unmasked key rows of k/v per batch (padded to a 128-multiple
SK; pad slots are masked out on device). For the ~50% random mask this nearly
halves all K-side work (K/V projection, scores, exp, context).

All x / W tensors travel as bf16 (host casts): half the DMA traffic, PE
transposes run at 1 cycle/row instead of 2, and no on-device f32->bf16 casts.

Per-core dataflow (everything stays transposed until the output projection):
  xT tiles (PE transpose, bf16) -> Q.T/K.T = W.T @ X.T, V natural (ones
  column appended) -> scores.T = K @ Q.T -> exp+mask+scale in one ACT op
  -> ctxU.T = V'.T @ expS.T (last row = softmax denominator) -> normalize
  (fast-approx reciprocal) -> out = ctx.T.T @ Wo + bo.
"""

import sys

if "/opt/trn_rl_repo" not in sys.path:
    sys.path.append("/opt/trn_rl_repo")

import numpy as np
import ml_dtypes

import concourse.bass as bass
import concourse.bacc as bacc
import concourse.tile as tile
from concourse import mybir
from concourse.bass import ts

F32 = mybir.dt.float32
BF16 = mybir.dt.bfloat16
I32 = mybir.dt.int32
EXP = mybir.ActivationFunctionType.Exp

P = 128
NPBF = ml_dtypes.bfloat16


def build_nc(S=2048, D=1024, DL=512, HD=64, SK=1152):
    """Per-core Bass program. DL = local out dim (heads*HD), SK = key len."""
    ST = S // P  # q token tiles
    SKT = SK // P  # key token tiles
    KD = D // P  # contraction tiles over D
    MT = DL // P  # local d-col tiles
    HL = DL // HD  # local heads
    HPT = P // HD  # heads per 128-partition tile (2)
    NCH = min(512, S)  # projection token-chunk
    QS = min(1024, S)  # attention q superchunk (<=2 psum banks)
    QH = min(512, QS)  # one-bank half
    NH = QS // QH
    NQ = S // QS  # q-superchunks
    OC = min(512, D)  # out-proj col chunk
    scale = float(1.0 / (np.sqrt(np.float32(HD)) + 1e-8))

    # K-side token chunks (SK may not be a NCH multiple)
    kchunks = []
    o = 0
    while o < SK:
        w_ = min(NCH, SK - o)
        kchunks.append((o, w_))
        o += w_

    nc = bacc.Bacc("TRN2", target_bir_lowering=False, debug=False)

    xq = nc.dram_tensor("xq", [S, D], BF16, kind="ExternalInput")
    xk = nc.dram_tensor("xk", [SK, D], BF16, kind="ExternalInput")
    xv = nc.dram_tensor("xv", [SK, D], BF16, kind="ExternalInput")
    msk = nc.dram_tensor("msk", [P, SKT], I32, kind="ExternalInput")
    wq = nc.dram_tensor("wq", [D, DL], BF16, kind="ExternalInput")
    wk = nc.dram_tensor("wk", [D, DL], BF16, kind="ExternalInput")
    wv = nc.dram_tensor("wv", [D, DL], BF16, kind="ExternalInput")
    wo = nc.dram_tensor("wo", [DL, D], BF16, kind="ExternalInput")
    bq = nc.dram_tensor("bq", [P, MT], F32, kind="ExternalInput")
    bk = nc.dram_tensor("bk", [P, MT], F32, kind="ExternalInput")
    bv = nc.dram_tensor("bv", [1, DL], F32, kind="ExternalInput")
    bo = nc.dram_tensor("bo", [1, D], F32, kind="ExternalInput")
    out = nc.dram_tensor("out", [S, D], F32, kind="ExternalOutput")

    with tile.TileContext(nc) as tc, nc.allow_low_precision("bf16 operands are rounded by design"):
        with (
            tc.tile_pool(name="pers", bufs=1) as pers,
            tc.tile_pool(name="wpool", bufs=2) as wpool,
            tc.tile_pool(name="exp", bufs=8) as ex_pool,
            tc.tile_pool(name="osb", bufs=3) as osb_pool,
            tc.tile_pool(name="small", bufs=2) as small,
        ):
            # ---- constants. All weight/bias/mask loads go through the
            # Scalar engine's DMA queue so the Sync queue (x transposes,
            # stores) is never blocked behind them. Everything except bks
            # is emitted after the first K-proj chunk so the first Sync
            # DMA is the one the critical path waits on. ----
            bks = pers.tile([P, MT], F32, tag="bks")
            nc.scalar.dma_start(bks[:], bk[:, :])

            mi = pers.tile([P, SKT], I32, tag="mi")
            mf = pers.tile([P, SKT], F32, tag="mf")
            mb = pers.tile([P, SKT], F32, tag="mb")
            bqs = pers.tile([P, MT], F32, tag="bqs")
            bvs = pers.tile([1, DL], F32, tag="bvs")
            bos = pers.tile([1, D], F32, tag="bos")
            bvb = pers.tile([P, DL], F32, tag="bvb")
            bob = pers.tile([P, D], F32, tag="bob")

            def late_consts():
                nc.scalar.dma_start(mi[:], msk[:, :])
                nc.vector.tensor_copy(out=mf[:], in_=mi[:])
                nc.vector.tensor_scalar_mul(mb[:], mf[:], -1.0e9)
                nc.scalar.dma_start(bqs[:], bq[:, :])
                nc.scalar.dma_start(bvs[:], bv[:, :])
                nc.scalar.dma_start(bos[:], bo[:, :])
                nc.gpsimd.partition_broadcast(bvb[:], bvs[0:1, :])
                nc.gpsimd.partition_broadcast(bob[:], bos[0:1, :])
                for t in range(SKT):
                    nc.gpsimd.memset(VP[t][:], 1.0)

            # persistent activation stores
            KT = [pers.tile([P, SK], BF16, tag=f"kt{m}", name=f"kt{m}") for m in range(MT)]
            QT = [pers.tile([P, S], BF16, tag=f"qt{m}", name=f"qt{m}") for m in range(MT)]
            CT = [pers.tile([P, S], BF16, tag=f"ct{m}", name=f"ct{m}") for m in range(MT)]
            VP = [pers.tile([P, HL * (HD + 1)], BF16, tag=f"vp{t}", name=f"vp{t}") for t in range(SKT)]

            # x.T via the DMA-engine transpose (XT[p, kk, t] = x[t, kk*128+p]):
            # no PE transposes, no PSUM staging. One instruction per tensor —
            # DMA-transposes barrier against other DMAs, so don't interleave.
            XK = pers.tile([P, KD, SK], BF16, tag="XK")
            XV = pers.tile([P, KD, SK], BF16, tag="XV")
            XQ = pers.tile([P, KD, S], BF16, tag="XQ")

            def load_w(wdram):
                w = wpool.tile([P, KD, DL], BF16, tag="w", name="w")
                nc.scalar.dma_start(w[:], wdram.rearrange("(k p) n -> p k n", p=P))
                return w

            wos = pers.tile([P, MT, D], BF16, tag="wos")

            def proj_mm_units(xt, wsb, bias_sb, dst_tiles, tok0, ntok, acc_pool):
                """dst[m][:, tok0:tok0+ntok] = (x @ w + b).T; yields per m."""
                for m in range(MT):
                    acc = acc_pool.tile([P, NCH], F32, tag="acc")
                    for kk in range(KD):
                        nc.tensor.matmul(
                            acc[:, 0:ntok],
                            lhsT=wsb[:, kk, ts(m, P)],
                            rhs=xt[:, kk, tok0 : tok0 + ntok],
                            start=(kk == 0),
                            stop=(kk == KD - 1),
                        )
                    nc.vector.tensor_scalar_add(
                        dst_tiles[m][:, tok0 : tok0 + ntok], acc[:, 0:ntok], bias_sb[:, m : m + 1]
                    )
                    yield

            def vproj(wsb, acc_pool):
                """VP[t][:, h*(HD+1):+HD] = (xv @ wv + bv)[t-tile, h-slice]."""
                for t in range(SKT):
                    acc = acc_pool.tile([P, DL], F32, tag="acc")
                    for kk in range(KD):
                        nc.tensor.matmul(
                            acc[:],
                            lhsT=XV[:, kk, ts(t, P)],
                            rhs=wsb[:, kk, :],
                            start=(kk == 0),
                            stop=(kk == KD - 1),
                        )
                    for h in range(HL):
                        nc.vector.tensor_add(
                            VP[t][:, h * (HD + 1) : h * (HD + 1) + HD],
                            acc[:, ts(h, HD)],
                            bvb[:, ts(h, HD)],
                        )

            def attention(qq, sc_pool, cx_pool, filler=None, pump_every=8):
                it = 0
                pending = []  # deferred normalize work (recip/broadcast/mul)
                for hp in range(HL // HPT):  # head pairs share a KT/QT tile
                    for q5 in range(NH):
                        col0 = qq * QS + q5 * QH
                        cxs = [
                            cx_pool.tile([HD + 1, QH], F32, tag="cx", name="cx")
                            for _ in range(HPT)
                        ]
                        for kt in range(SKT):
                            # one PSUM supertile holds both heads' score chunk;
                            # the two K=64 matmuls run concurrently (row groups
                            # 0-63 / 64-127), one ACT exp covers both
                            sc = sc_pool.tile([P, HPT * QH], F32, tag="sc")
                            for u in range(HPT):
                                mo = u * HD
                                nc.tensor.matmul(
                                    sc[:, ts(u, QH)],
                                    lhsT=KT[hp][mo : mo + HD, ts(kt, P)],
                                    rhs=QT[hp][mo : mo + HD, col0 : col0 + QH],
                                    start=True,
                                    stop=True,
                                )
                            ex = ex_pool.tile([P, HPT * QH], BF16, tag="ex")
                            nc.scalar.activation(
                                ex[:], sc[:], EXP, bias=mb[:, kt : kt + 1], scale=scale
                            )
                            for u in range(HPT):
                                h = hp * HPT + u
                                nc.tensor.matmul(
                                    cxs[u][:],
                                    lhsT=VP[kt][:, h * (HD + 1) : (h + 1) * (HD + 1)],
                                    rhs=ex[:, ts(u, QH)],
                                    start=(kt == 0),
                                    stop=(kt == SKT - 1),
                                )
                            it += 1
                            if filler is not None and it % pump_every == 0:
                                next(filler, None)
                        prev_tails = pending
                        pending = []
                        for u in range(HPT):
                            mo = u * HD
                            # the cheap DVE copy (emitted now, ahead of the
                            # previous unit's reciprocals in DVE order) frees
                            # the ctx PSUM slot; recip/broadcast/mul are
                            # deferred one unit so nothing waits on them
                            stg = small.tile([HD + 1, QH], F32, tag="stg", name="stg", bufs=4)
                            nc.vector.tensor_copy(out=stg[:], in_=cxs[u][:])

                            def tail(hp=hp, mo=mo, col0=col0, stg=stg):
                                # denominator row lives at partition HD; the
                                # custom gpsimd/DVE ops read absolute partition
                                # 0 on HW, so DMA it down to a base-0 tile first
                                rec = small.tile([1, QH], F32, tag="rec", name="rec", bufs=2)
                                nc.sync.dma_start(rec[0:1, :], stg[HD : HD + 1, :])
                                dnb = small.tile([HD, QH], F32, tag="dnb", name="dnb", bufs=2)
                                nc.gpsimd.partition_broadcast(dnb[:], rec[0:1, :])
                                bcs = small.tile([HD, QH], F32, tag="bcs", bufs=2)
                                nc.vector.reciprocal_approx_fast(bcs[:], dnb[:])
                                if mo == 0:
                                    nc.vector.tensor_mul(
                                        CT[hp][0:HD, col0 : col0 + QH],
                                        stg[0:HD, :],
                                        bcs[:],
                                    )
                                else:
                                    tmp = small.tile([HD, QH], BF16, tag="tmp")
                                    nc.vector.tensor_mul(tmp[:], stg[0:HD, :], bcs[:])
                                    nc.sync.dma_start(
                                        CT[hp][mo : mo + HD, col0 : col0 + QH], tmp[:]
                                    )

                            pending.append(tail)
                        for fn in prev_tails:
                            fn()

                for fn in pending:
                    fn()

            def outproj_units(qq, acc_pool):
                t0 = qq * (QS // P)
                for t in range(t0, t0 + QS // P):
                    for c in range(D // OC):
                        po = acc_pool.tile([P, OC], F32, tag="acc", name="po")
                        for dd in range(MT):
                            nc.tensor.matmul(
                                po[:],
                                lhsT=CT[dd][:, ts(t, P)],
                                rhs=wos[:, dd, ts(c, OC)],
                                start=(dd == 0),
                                stop=(dd == MT - 1),
                            )
                        osb = osb_pool.tile([P, OC], F32, tag="osb")
                        nc.vector.tensor_add(osb[:], po[:], bob[:, ts(c, OC)])
                        nc.sync.dma_start(out[ts(t, P), ts(c, OC)], osb[:])
                        yield

            CPQ = QS // NCH  # projection chunks per q-superchunk

            # ---- phase 1: K.T and V' (full-S prerequisites of attention) ----
            with tc.tile_pool(name="ps1acc", bufs=6, space="PSUM") as ps1acc:
                wks = load_w(wk)
                wvs = load_w(wv)
                nc.sync.dma_start_transpose(XK[:], xk[:, :])
                nc.sync.dma_start_transpose(XV[:], xv[:, :])
                nc.sync.dma_start_transpose(XQ[:], xq[:, :])
                for ci, (tok0, ntok) in enumerate(kchunks):
                    for _ in proj_mm_units(XK, wks, bks, KT, tok0, ntok, ps1acc):
                        pass
                    if ci == 0:
                        late_consts()
                vproj(wvs, ps1acc)
                wqs = load_w(wq)
                nc.scalar.dma_start(wos[:], wo.rearrange("(m p) n -> p m n", p=P))

            # ---- phase 2: Q.T chunks, attention, out-proj ----
            with (
                tc.tile_pool(name="ps2acc", bufs=2, space="PSUM") as ps2acc,
                tc.tile_pool(name="ps2sc", bufs=2, space="PSUM") as ps2sc,
                tc.tile_pool(name="ps2cx", bufs=2, space="PSUM") as ps2cx,
            ):
                from itertools import chain

                n_att_its = (HL // HPT) * NH * SKT
                for nch in range(CPQ):
                    for _ in proj_mm_units(
                        XQ, wqs, bqs, QT, nch * NCH, NCH, ps2acc
                    ):
                        pass
                for qq in range(NQ):
                    if qq + 1 < NQ:
                        filler = chain.from_iterable(
                            proj_mm_units(XQ, wqs, bqs, QT, nch * NCH, NCH, ps2acc)
                            for nch in range((qq + 1) * CPQ, (qq + 2) * CPQ)
                        )
                        n_units = CPQ * MT
                    elif qq >= 1:
                        filler = outproj_units(qq - 1, ps2acc)
                        n_units = (QS // P) * (D // OC)
                    else:
                        filler = None
                        n_units = 1
                    attention(
                        qq,
                        ps2sc,
                        ps2cx,
                        filler,
                        pump_every=max(1, n_att_its // max(n_units, 1)),
                    )
                    if filler is not None:
                        for _ in filler:
                            pass
                if NQ >= 2:
                    for qq in range(NQ - 2):
                        for _ in outproj_units(qq, ps2acc):
                            pass

            # ---- phase 3: final out-proj; psum depth 2 is enough for the
            # epilogue (vector add + store) to trail by less than a unit ----
            with tc.tile_pool(name="ps3", bufs=2, space="PSUM") as ps3:
                for _ in outproj_units(NQ - 1, ps3):
                    pass

    nc.compile()
    return nc


_NC_CACHE = {}


def _get_nc(S, D, DL, HD, SK):
    key = (S, D, DL, HD, SK)
    if key not in _NC_CACHE:
        _NC_CACHE[key] = build_nc(S, D, DL, HD, SK)
    return _NC_CACHE[key]


def _shard_inputs(q, k, v, mask, Wq, bq, Wk, bk, Wv, bv, Wo, bo):
    q, k, v = np.asarray(q), np.asarray(k), np.asarray(v)
    mask = np.asarray(mask)
    Wq, Wk, Wv, Wo = np.asarray(Wq), np.asarray(Wk), np.asarray(Wv), np.asarray(Wo)
    bq, bk, bv, bo = np.asarray(bq), np.asarray(bk), np.asarray(bv), np.asarray(bo)

    B, S, D = q.shape  # 4, 2048, 1024
    G = 2  # head-groups (tensor-parallel factor); B*G = 8 cores
    DL = D // G
    MT = DL // P

    # key compaction: gather unmasked key rows, pad to a 128-multiple
    keep = [np.flatnonzero(mask[b, 0, 0] == 0) for b in range(B)]
    maxc = max(max((len(ix) for ix in keep), default=1), 1)
    SK = min(S, ((maxc + P - 1) // P) * P)
    SKT = SK // P

    f32 = np.float32
    xk_c, xv_c, msk_c = [], [], []
    for b in range(B):
        if SK == S:
            # fallback: no compaction, original order + original mask
            xk_c.append(np.ascontiguousarray(k[b], dtype=NPBF))
            xv_c.append(np.ascontiguousarray(v[b], dtype=NPBF))
            msk_c.append(
                np.ascontiguousarray(mask[b, 0, 0].reshape(SKT, P).T, dtype=np.int32)
            )
        else:
            ix = keep[b][:SK]
            n = len(ix)
            kb = np.zeros((SK, D), dtype=NPBF)
            vb = np.zeros((SK, D), dtype=NPBF)
            kb[:n] = k[b][ix].astype(NPBF)
            vb[:n] = v[b][ix].astype(NPBF)
            mb_ = np.zeros((SK,), dtype=np.int32)
            mb_[n:] = 1
            xk_c.append(kb)
            xv_c.append(vb)
            msk_c.append(np.ascontiguousarray(mb_.reshape(SKT, P).T, dtype=np.int32))

    xq_b = [np.ascontiguousarray(q[b], dtype=NPBF) for b in range(B)]

    in_maps = []
    for c in range(B * G):
        b, g = c // G, c % G
        sl = slice(g * DL, (g + 1) * DL)
        bo_core = bo if g == 0 else np.zeros_like(bo)
        in_maps.append(
            {
                "xq": xq_b[b],
                "xk": xk_c[b],
                "xv": xv_c[b],
                "msk": msk_c[b],
                "wq": np.ascontiguousarray(Wq[:, sl].astype(NPBF)),
                "wk": np.ascontiguousarray(Wk[:, sl].astype(NPBF)),
                "wv": np.ascontiguousarray(Wv[:, sl].astype(NPBF)),
                "wo": np.ascontiguousarray(Wo[sl, :].astype(NPBF)),
                "bq": np.ascontiguousarray(bq[sl].reshape(MT, P).T, dtype=f32),
                "bk": np.ascontiguousarray(bk[sl].reshape(MT, P).T, dtype=f32),
                "bv": np.ascontiguousarray(bv[sl].reshape(1, DL), dtype=f32),
                "bo": np.ascontiguousarray(bo_core.reshape(1, D), dtype=f32),
            }
        )
    return in_maps, SK


def kernel(q, k, v, mask, Wq, bq, Wk, bk, Wv, bv, Wo, bo):
    from concourse.bass_utils import run_bass_kernel_spmd

    q = np.asarray(q)
    B, S, D = q.shape  # 4, 2048, 1024
    G = 2
    in_maps, SK = _shard_inputs(q, k, v, mask, Wq, bq, Wk, bk, Wv, bv, Wo, bo)
    nc = _get_nc(S, D, D // G, 64, SK)

    res = run_bass_kernel_spmd(nc, in_maps, core_ids=list(range(B * G)))
    parts = [r["out"] for r in res.results]
    outf = np.stack([parts[b * G] + parts[b * G + 1] for b in range(B)], axis=0)
    return outf.astype(np.float32)


# revision 20
# speedup vs baseline: 2.2717x; 1.0184x over previous
"""Multi-head attention (B=4, S=2048, D=1024, H=16) on 8 trn2 NeuronCores.

Sharding: 8 cores = 4 batches x 2 head-groups. Core c handles batch c//2 and
heads [8g, 8g+8) where g = c%2 (tensor-parallel: Wq/Wk/Wv column-sliced,
Wo row-sliced). Each core returns a partial output [S, D]; the host sums the
two head-group partials per batch.

Key-compaction: the mask drops a key entirely (exp(-1e9) == 0), so the host
gathers only the unmasked key rows of k/v per batch (padded to a 128-multiple
SK; pad slots are masked out on device). For the ~50% random mask this nearly
halves all K-side work (K/V projection, scores, exp, context).

All x / W tensors travel as bf16 (host casts): half the DMA traffic, PE
transposes run at 1 cycle/row instead of 2, and no on-device f32->bf16 casts.

Per-core dataflow (everything stays transposed until the output projection):
  xT tiles (PE transpose, bf16) -> Q.T/K.T = W.T @ X.T, V natural (ones
  column appended) -> scores.T = K @ Q.T -> exp+mask+scale in one ACT op
  -> ctxU.T = V'.T @ expS.T (last row = softmax denominator) -> normalize
  (fast-approx reciprocal) -> out = ctx.T.T @ Wo + bo.
"""

import sys

if "/opt/trn_rl_repo" not in sys.path:
    sys.path.append("/opt/trn_rl_repo")

import numpy as np
import ml_dtypes

import concourse.bass as bass
import concourse.bacc as bacc
import concourse.tile as tile
from concourse import mybir
from concourse.bass import ts

F32 = mybir.dt.float32
BF16 = mybir.dt.bfloat16
I32 = mybir.dt.int32
EXP = mybir.ActivationFunctionType.Exp

P = 128
NPBF = ml_dtypes.bfloat16


def build_nc(S=2048, D=1024, DL=512, HD=64, SK=1152):
    """Per-core Bass program. DL = local out dim (heads*HD), SK = key len."""
    ST = S // P  # q token tiles
    SKT = SK // P  # key token tiles
    KD = D // P  # contraction tiles over D
    MT = DL // P  # local d-col tiles
    HL = DL // HD  # local heads
    HPT = P // HD  # heads per 128-partition tile (2)
    NCH = min(512, S)  # projection token-chunk
    QS = min(1024, S)  # attention q superchunk (<=2 psum banks)
    QH = min(512, QS)  # one-bank half
    NH = QS // QH
    NQ = S // QS  # q-superchunks
    OC = min(512, D)  # out-proj col chunk
    scale = float(1.0 / (np.sqrt(np.float32(HD)) + 1e-8))

    # K-side token chunks (SK may not be a NCH multiple)
    kchunks = []
    o = 0
    while o < SK:
        w_ = min(NCH, SK - o)
        kchunks.append((o, w_))
        o += w_

    nc = bacc.Bacc("TRN2", target_bir_lowering=False, debug=False)

    xq = nc.dram_tensor("xq", [S, D], BF16, kind="ExternalInput")
    xk = nc.dram_tensor("xk", [SK, D], BF16, kind="ExternalInput")
    xv = nc.dram_tensor("xv", [SK, D], BF16, kind="ExternalInput")
    msk = nc.dram_tensor("msk", [P, SKT], I32, kind="ExternalInput")
    wq = nc.dram_tensor("wq", [D, DL], BF16, kind="ExternalInput")
    wk = nc.dram_tensor("wk", [D, DL], BF16, kind="ExternalInput")
    wv = nc.dram_tensor("wv", [D, DL], BF16, kind="ExternalInput")
    wo = nc.dram_tensor("wo", [DL, D], BF16, kind="ExternalInput")
    bq = nc.dram_tensor("bq", [P, MT], F32, kind="ExternalInput")
    bk = nc.dram_tensor("bk", [P, MT], F32, kind="ExternalInput")
    bv = nc.dram_tensor("bv", [1, DL], F32, kind="ExternalInput")
    bo = nc.dram_tensor("bo", [1, D], F32, kind="ExternalInput")
    out = nc.dram_tensor("out", [S, D], F32, kind="ExternalOutput")

    with tile.TileContext(nc) as tc, nc.allow_low_precision("bf16 operands are rounded by design"):
        with (
            tc.tile_pool(name="pers", bufs=1) as pers,
            tc.tile_pool(name="wpool", bufs=2) as wpool,
            tc.tile_pool(name="exp", bufs=8) as ex_pool,
            tc.tile_pool(name="osb", bufs=3) as osb_pool,
            tc.tile_pool(name="small", bufs=2) as small,
        ):
            # ---- constants. All weight/bias/mask loads go through the
            # Scalar engine's DMA queue so the Sync queue (x transposes,
            # stores) is never blocked behind them. Everything except bks
            # is emitted after the first K-proj chunk so the first Sync
            # DMA is the one the critical path waits on. ----
            bks = pers.tile([P, MT], F32, tag="bks")
            nc.scalar.dma_start(bks[:], bk[:, :])

            mi = pers.tile([P, SKT], I32, tag="mi")
            mf = pers.tile([P, SKT], F32, tag="mf")
            mb = pers.tile([P, SKT], F32, tag="mb")
            bqs = pers.tile([P, MT], F32, tag="bqs")
            bvs = pers.tile([1, DL], F32, tag="bvs")
            bos = pers.tile([1, D], F32, tag="bos")
            bvb = pers.tile([P, DL], F32, tag="bvb")
            bob = pers.tile([P, D], F32, tag="bob")

            def late_consts():
                nc.scalar.dma_start(mi[:], msk[:, :])
                nc.vector.tensor_copy(out=mf[:], in_=mi[:])
                nc.vector.tensor_scalar_mul(mb[:], mf[:], -1.0e9)
                nc.scalar.dma_start(bqs[:], bq[:, :])
                nc.scalar.dma_start(bvs[:], bv[:, :])
                nc.scalar.dma_start(bos[:], bo[:, :])
                nc.gpsimd.partition_broadcast(bvb[:], bvs[0:1, :])
                nc.gpsimd.partition_broadcast(bob[:], bos[0:1, :])
                for t in range(SKT):
                    nc.gpsimd.memset(VP[t][:], 1.0)

            # persistent activation stores
            KT = [pers.tile([P, SK], BF16, tag=f"kt{m}", name=f"kt{m}") for m in range(MT)]
            QT = [pers.tile([P, S], BF16, tag=f"qt{m}", name=f"qt{m}") for m in range(MT)]
            CT = [pers.tile([P, S], BF16, tag=f"ct{m}", name=f"ct{m}") for m in range(MT)]
            VP = [pers.tile([P, HL * (HD + 1)], BF16, tag=f"vp{t}", name=f"vp{t}") for t in range(SKT)]

            # x.T via the DMA-engine transpose (XT[p, kk, t] = x[t, kk*128+p]):
            # no PE transposes, no PSUM staging. One instruction per tensor —
            # DMA-transposes barrier against other DMAs, so don't interleave.
            XK = pers.tile([P, KD, SK], BF16, tag="XK")
            XV = pers.tile([P, KD, SK], BF16, tag="XV")
            XQ = pers.tile([P, KD, S], BF16, tag="XQ")

            def load_w(wdram):
                w = wpool.tile([P, KD, DL], BF16, tag="w", name="w")
                nc.scalar.dma_start(w[:], wdram.rearrange("(k p) n -> p k n", p=P))
                return w

            wos = pers.tile([P, MT, D], BF16, tag="wos")

            def proj_mm_units(xt, wsb, bias_sb, dst_tiles, tok0, ntok, acc_pool):
                """dst[m][:, tok0:tok0+ntok] = (x @ w + b).T; yields per m."""
                for m in range(MT):
                    acc = acc_pool.tile([P, NCH], F32, tag="acc")
                    for kk in range(KD):
                        nc.tensor.matmul(
                            acc[:, 0:ntok],
                            lhsT=wsb[:, kk, ts(m, P)],
                            rhs=xt[:, kk, tok0 : tok0 + ntok],
                            start=(kk == 0),
                            stop=(kk == KD - 1),
                        )
                    nc.vector.tensor_scalar_add(
                        dst_tiles[m][:, tok0 : tok0 + ntok], acc[:, 0:ntok], bias_sb[:, m : m + 1]
                    )
                    yield

            def vproj(wsb, acc_pool):
                """VP[t][:, h*(HD+1):+HD] = (xv @ wv + bv)[t-tile, h-slice]."""
                for t in range(SKT):
                    acc = acc_pool.tile([P, DL], F32, tag="acc")
                    for kk in range(KD):
                        nc.tensor.matmul(
                            acc[:],
                            lhsT=XV[:, kk, ts(t, P)],
                            rhs=wsb[:, kk, :],
                            start=(kk == 0),
                            stop=(kk == KD - 1),
                        )
                    for h in range(HL):
                        nc.vector.tensor_add(
                            VP[t][:, h * (HD + 1) : h * (HD + 1) + HD],
                            acc[:, ts(h, HD)],
                            bvb[:, ts(h, HD)],
                        )

            def attention(qq, sc_pool, cx_pool, filler=None, pump_every=8):
                it = 0
                pending = []  # deferred normalize work (recip/broadcast/mul)
                for hp in range(HL // HPT):  # head pairs share a KT/QT tile
                    for q5 in range(NH):
                        col0 = qq * QS + q5 * QH
                        cxs = [
                            cx_pool.tile([HD + 1, QH], F32, tag="cx", name="cx")
                            for _ in range(HPT)
                        ]
                        for kt in range(SKT):
                            # one PSUM supertile holds both heads' score chunk;
                            # the two K=64 matmuls run concurrently (row groups
                            # 0-63 / 64-127), one ACT exp covers both
                            sc = sc_pool.tile([P, HPT * QH], F32, tag="sc")
                            for u in range(HPT):
                                mo = u * HD
                                nc.tensor.matmul(
                                    sc[:, ts(u, QH)],
                                    lhsT=KT[hp][mo : mo + HD, ts(kt, P)],
                                    rhs=QT[hp][mo : mo + HD, col0 : col0 + QH],
                                    start=True,
                                    stop=True,
                                )
                            ex = ex_pool.tile([P, HPT * QH], BF16, tag="ex")
                            nc.scalar.activation(
                                ex[:], sc[:], EXP, bias=mb[:, kt : kt + 1], scale=scale
                            )
                            for u in range(HPT):
                                h = hp * HPT + u
                                nc.tensor.matmul(
                                    cxs[u][:],
                                    lhsT=VP[kt][:, h * (HD + 1) : (h + 1) * (HD + 1)],
                                    rhs=ex[:, ts(u, QH)],
                                    start=(kt == 0),
                                    stop=(kt == SKT - 1),
                                )
                            it += 1
                            if filler is not None and it % pump_every == 0:
                                next(filler, None)
                        prev_tails = pending
                        pending = []
                        for u in range(HPT):
                            mo = u * HD
                            # the cheap DVE copy (emitted now, ahead of the
                            # previous unit's reciprocals in DVE order) frees
                            # the ctx PSUM slot; recip/broadcast/mul are
                            # deferred one unit so nothing waits on them
                            stg = small.tile([HD + 1, QH], F32, tag="stg", name="stg", bufs=4)
                            nc.vector.tensor_copy(out=stg[:], in_=cxs[u][:])

                            def tail(hp=hp, mo=mo, col0=col0, stg=stg):
                                # denominator row lives at partition HD; the
                                # custom gpsimd/DVE ops read absolute partition
                                # 0 on HW, so DMA it down to a base-0 tile first
                                rec = small.tile([1, QH], F32, tag="rec", name="rec", bufs=2)
                                nc.sync.dma_start(rec[0:1, :], stg[HD : HD + 1, :])
                                dnb = small.tile([HD, QH], F32, tag="dnb", name="dnb", bufs=2)
                                nc.gpsimd.partition_broadcast(dnb[:], rec[0:1, :])
                                bcs = small.tile([HD, QH], F32, tag="bcs", bufs=2)
                                nc.vector.reciprocal_approx_fast(bcs[:], dnb[:])
                                if mo == 0:
                                    nc.vector.tensor_mul(
                                        CT[hp][0:HD, col0 : col0 + QH],
                                        stg[0:HD, :],
                                        bcs[:],
                                    )
                                else:
                                    tmp = small.tile([HD, QH], BF16, tag="tmp")
                                    nc.vector.tensor_mul(tmp[:], stg[0:HD, :], bcs[:])
                                    nc.sync.dma_start(
                                        CT[hp][mo : mo + HD, col0 : col0 + QH], tmp[:]
                                    )

                            pending.append(tail)
                        for fn in prev_tails:
                            fn()

                for fn in pending:
                    fn()

            def outproj_units(qq, acc_pool):
                t0 = qq * (QS // P)
                for t in range(t0, t0 + QS // P):
                    for c in range(D // OC):
                        po = acc_pool.tile([P, OC], F32, tag="acc", name="po")
                        for dd in range(MT):
                            nc.tensor.matmul(
                                po[:],
                                lhsT=CT[dd][:, ts(t, P)],
                                rhs=wos[:, dd, ts(c, OC)],
                                start=(dd == 0),
                                stop=(dd == MT - 1),
                            )
                        osb = osb_pool.tile([P, OC], F32, tag="osb")
                        nc.vector.tensor_add(osb[:], po[:], bob[:, ts(c, OC)])
                        nc.sync.dma_start(out[ts(t, P), ts(c, OC)], osb[:])
                        yield

            CPQ = QS // NCH  # projection chunks per q-superchunk

            # ---- phase 1: K.T and V' (full-S prerequisites of attention) ----
            # DMA-transposes barrier against every other DMA (each waits all
            # prior completions and blocks later ones), so the whole DMA
            # program is ordered by when each transfer is actually needed:
            # XK -> wk/wv -> XV -> consts -> XQ -> wq/wo.
            with tc.tile_pool(name="ps1acc", bufs=6, space="PSUM") as ps1acc:
                nc.sync.dma_start_transpose(XK[:], xk[:, :])
                wks = load_w(wk)
                wvs = load_w(wv)
                nc.sync.dma_start_transpose(XV[:], xv[:, :])
                late_consts()
                nc.sync.dma_start_transpose(XQ[:], xq[:, :])
                wqs = load_w(wq)
                nc.scalar.dma_start(wos[:], wo.rearrange("(m p) n -> p m n", p=P))
                for tok0, ntok in kchunks:
                    for _ in proj_mm_units(XK, wks, bks, KT, tok0, ntok, ps1acc):
                        pass
                vproj(wvs, ps1acc)

            # ---- phase 2: Q.T chunks, attention, out-proj ----
            with (
                tc.tile_pool(name="ps2acc", bufs=2, space="PSUM") as ps2acc,
                tc.tile_pool(name="ps2sc", bufs=2, space="PSUM") as ps2sc,
                tc.tile_pool(name="ps2cx", bufs=2, space="PSUM") as ps2cx,
            ):
                from itertools import chain

                n_att_its = (HL // HPT) * NH * SKT
                for nch in range(CPQ):
                    for _ in proj_mm_units(
                        XQ, wqs, bqs, QT, nch * NCH, NCH, ps2acc
                    ):
                        pass
                for qq in range(NQ):
                    if qq + 1 < NQ:
                        filler = chain.from_iterable(
                            proj_mm_units(XQ, wqs, bqs, QT, nch * NCH, NCH, ps2acc)
                            for nch in range((qq + 1) * CPQ, (qq + 2) * CPQ)
                        )
                        n_units = CPQ * MT
                    elif qq >= 1:
                        filler = outproj_units(qq - 1, ps2acc)
                        n_units = (QS // P) * (D // OC)
                    else:
                        filler = None
                        n_units = 1
                    attention(
                        qq,
                        ps2sc,
                        ps2cx,
                        filler,
                        pump_every=max(1, n_att_its // max(n_units, 1)),
                    )
                    if filler is not None:
                        for _ in filler:
                            pass
                if NQ >= 2:
                    for qq in range(NQ - 2):
                        for _ in outproj_units(qq, ps2acc):
                            pass

            # ---- phase 3: final out-proj; psum depth 2 is enough for the
            # epilogue (vector add + store) to trail by less than a unit ----
            with tc.tile_pool(name="ps3", bufs=2, space="PSUM") as ps3:
                for _ in outproj_units(NQ - 1, ps3):
                    pass

    nc.compile()
    return nc


_NC_CACHE = {}


def _get_nc(S, D, DL, HD, SK):
    key = (S, D, DL, HD, SK)
    if key not in _NC_CACHE:
        _NC_CACHE[key] = build_nc(S, D, DL, HD, SK)
    return _NC_CACHE[key]


def _shard_inputs(q, k, v, mask, Wq, bq, Wk, bk, Wv, bv, Wo, bo):
    q, k, v = np.asarray(q), np.asarray(k), np.asarray(v)
    mask = np.asarray(mask)
    Wq, Wk, Wv, Wo = np.asarray(Wq), np.asarray(Wk), np.asarray(Wv), np.asarray(Wo)
    bq, bk, bv, bo = np.asarray(bq), np.asarray(bk), np.asarray(bv), np.asarray(bo)

    B, S, D = q.shape  # 4, 2048, 1024
    G = 2  # head-groups (tensor-parallel factor); B*G = 8 cores
    DL = D // G
    MT = DL // P

    # key compaction: gather unmasked key rows, pad to a 128-multiple
    keep = [np.flatnonzero(mask[b, 0, 0] == 0) for b in range(B)]
    maxc = max(max((len(ix) for ix in keep), default=1), 1)
    SK = min(S, ((maxc + P - 1) // P) * P)
    SKT = SK // P

    f32 = np.float32
    xk_c, xv_c, msk_c = [], [], []
    for b in range(B):
        if SK == S:
            # fallback: no compaction, original order + original mask
            xk_c.append(np.ascontiguousarray(k[b], dtype=NPBF))
            xv_c.append(np.ascontiguousarray(v[b], dtype=NPBF))
            msk_c.append(
                np.ascontiguousarray(mask[b, 0, 0].reshape(SKT, P).T, dtype=np.int32)
            )
        else:
            ix = keep[b][:SK]
            n = len(ix)
            kb = np.zeros((SK, D), dtype=NPBF)
            vb = np.zeros((SK, D), dtype=NPBF)
            kb[:n] = k[b][ix].astype(NPBF)
            vb[:n] = v[b][ix].astype(NPBF)
            mb_ = np.zeros((SK,), dtype=np.int32)
            mb_[n:] = 1
            xk_c.append(kb)
            xv_c.append(vb)
            msk_c.append(np.ascontiguousarray(mb_.reshape(SKT, P).T, dtype=np.int32))

    xq_b = [np.ascontiguousarray(q[b], dtype=NPBF) for b in range(B)]

    in_maps = []
    for c in range(B * G):
        b, g = c // G, c % G
        sl = slice(g * DL, (g + 1) * DL)
        bo_core = bo if g == 0 else np.zeros_like(bo)
        in_maps.append(
            {
                "xq": xq_b[b],
                "xk": xk_c[b],
                "xv": xv_c[b],
                "msk": msk_c[b],
                "wq": np.ascontiguousarray(Wq[:, sl].astype(NPBF)),
                "wk": np.ascontiguousarray(Wk[:, sl].astype(NPBF)),
                "wv": np.ascontiguousarray(Wv[:, sl].astype(NPBF)),
                "wo": np.ascontiguousarray(Wo[sl, :].astype(NPBF)),
                "bq": np.ascontiguousarray(bq[sl].reshape(MT, P).T, dtype=f32),
                "bk": np.ascontiguousarray(bk[sl].reshape(MT, P).T, dtype=f32),
                "bv": np.ascontiguousarray(bv[sl].reshape(1, DL), dtype=f32),
                "bo": np.ascontiguousarray(bo_core.reshape(1, D), dtype=f32),
            }
        )
    return in_maps, SK


def kernel(q, k, v, mask, Wq, bq, Wk, bk, Wv, bv, Wo, bo):
    from concourse.bass_utils import run_bass_kernel_spmd

    q = np.asarray(q)
    B, S, D = q.shape  # 4, 2048, 1024
    G = 2
    in_maps, SK = _shard_inputs(q, k, v, mask, Wq, bq, Wk, bk, Wv, bv, Wo, bo)
    nc = _get_nc(S, D, D // G, 64, SK)

    res = run_bass_kernel_spmd(nc, in_maps, core_ids=list(range(B * G)))
    parts = [r["out"] for r in res.results]
    outf = np.stack([parts[b * G] + parts[b * G + 1] for b in range(B)], axis=0)
    return outf.astype(np.float32)
